# revision 25
# baseline (speedup 1.0000x reference)
"""Trainium2 Bass kernel for the DSA-MoE routing module.

Strategy: data-parallel over batch. Each of the 8 NeuronCores gets 2 full
batches (4096 tokens). Expert weights are replicated, cast to bf16 on host,
and kept SBUF-resident. Per core:

  - cond router: sum features over seq via DVE reduce on the pre-transposed
    bf16 activations, then a small fp32 matmul against cond_w/2048 (+cond_b
    via a rank-1 matmul). Final softmax happens on host from the returned
    logits; an on-device softmax produces the internal routing weights.
  - stage router: x_raw is tiny (16x2048x14) -> host computes stage softmax
    and passes the per-batch weights in.
  - main pipeline per 256-token block: GEMM1 (X.T @ W_down -> H.T in
    [EH, tokens] layout, bf16, PSUM fp32), fused Gelu+down_b eviction on
    ACT, per-(batch,expert) scale on DVE, GEMM2 accumulates
    delta = Hw.T^T @ W_up directly in PSUM across all 9 EH-chunks
    (plus a rank-1 matmul adding the up_b bias term), final eviction adds
    the fp32 residual.

The full (unsharded) inputs come in; sharding/gather happens on host.
"""

import sys

sys.path.insert(0, "/opt/trn_rl_repo")

from contextlib import ExitStack

import ml_dtypes
import numpy as np

import concourse.bass as bass  # noqa: F401  (registers bass types)
import concourse.tile as tile
from concourse import bacc, mybir
from concourse.bass_utils import run_bass_kernel_spmd

BF16, F32 = mybir.dt.bfloat16, mybir.dt.float32
FP8 = mybir.dt.float8e4
AF = mybir.ActivationFunctionType
AX = mybir.AxisListType
ALU = mybir.AluOpType
BF16_NP = ml_dtypes.bfloat16
FP8_NP = ml_dtypes.float8_e4m3

# fp8 GEMM2: h is pre-scaled by HS (off the e4m3 subnormal range) and W_up by
# WS; the final eviction divides the PSUM result by HS*WS, with the residual
# and bias terms pre-multiplied to match.
G2_FP8 = False
HS = 128.0
WS = 64.0
TOT = HS * WS

B, S, DM, HID = 16, 2048, 1024, 256
C, G = 6, 3
E = C * G                      # 18 experts
EH = E * HID                   # 4608
NCORE = 8
BPC = B // NCORE               # batches per core = 2
TPC = BPC * S                  # tokens per core = 4096
TB = 256                       # token block
NBLK = S // TB                 # blocks per batch = 8
KD = DM // 128                 # 8 k-tiles over D
NCH = 9                        # EH chunks
CHW = EH // NCH                # 512 EH cols per chunk
NKEH = EH // 128               # 36 EH k-tiles

_CACHE = {}
LAST_RESULT = None


def _build():
    nc = bacc.Bacc("TRN2", target_bir_lowering=False, debug=False,
                   num_devices=NCORE)
    xt = nc.dram_tensor("xt", [DM, TPC], BF16, kind="ExternalInput").ap()
    xres = nc.dram_tensor("xres", [TPC, DM], F32, kind="ExternalInput").ap()
    wdn = nc.dram_tensor("wdn", [DM, EH], BF16, kind="ExternalInput").ap()
    wup = nc.dram_tensor("wup", [EH, DM], FP8 if G2_FP8 else BF16,
                         kind="ExternalInput").ap()
    dbt = nc.dram_tensor("dbt", [128, NKEH], F32, kind="ExternalInput").ap()
    upb = nc.dram_tensor("upb", [E, DM], F32, kind="ExternalInput").ap()
    cws = nc.dram_tensor("cws", [DM, C], F32, kind="ExternalInput").ap()
    cb = nc.dram_tensor("cb", [1, C], F32, kind="ExternalInput").ap()
    sw = nc.dram_tensor("sw", [1, BPC * G], F32, kind="ExternalInput").ap()
    out = nc.dram_tensor("out", [TPC, DM], F32, kind="ExternalOutput").ap()
    clg = nc.dram_tensor("clg", [BPC, C], F32, kind="ExternalOutput").ap()

    with tile.TileContext(nc) as tc, ExitStack() as ctx:
        const = ctx.enter_context(tc.tile_pool(name="const", bufs=1))
        wpool = ctx.enter_context(tc.tile_pool(name="wpool", bufs=1))
        xtp = ctx.enter_context(tc.tile_pool(name="xtp", bufs=2))
        xrp = ctx.enter_context(tc.tile_pool(name="xrp", bufs=1))
        htp = ctx.enter_context(tc.tile_pool(name="htp", bufs=6))
        outp = ctx.enter_context(tc.tile_pool(name="outp", bufs=1))
        ps_g1 = ctx.enter_context(tc.tile_pool(name="psg1", bufs=4, space="PSUM"))
        ps_d = ctx.enter_context(tc.tile_pool(name="psd", bufs=1, space="PSUM"))
        ps_s = ps_g1  # router PSUM borrows the G1 bank ring (tag "g1")

        # DMA issue order is roughly execution order: wdn + block-0
        # activations first (gives PE its runway), then the router means,
        # then wup (trickles in under block 0's GEMM1), then the rest.
        wdn_sb = wpool.tile([128, KD * EH], BF16)       # k-tile k at [:, k*EH:]
        for k in range(KD):
            nc.sync.dma_start(out=wdn_sb[:, k * EH:(k + 1) * EH],
                              in_=wdn[k * 128:(k + 1) * 128, :])
        xtb0 = xtp.tile([128, KD * TB], BF16, tag="xtm", name="xtb0")
        for k in range(KD):
            nc.sync.dma_start(out=xtb0[:, k * TB:(k + 1) * TB],
                              in_=xt[k * 128:(k + 1) * 128, 0:TB])
        dbt_sb = const.tile([128, NKEH], F32)
        nc.sync.dma_start(out=dbt_sb[:], in_=dbt[:])
        cws_sb = const.tile([128, KD * C], F32)
        for k in range(KD):
            nc.sync.dma_start(out=cws_sb[:, k * C:(k + 1) * C],
                              in_=cws[k * 128:(k + 1) * 128, :])
        cb_sb = const.tile([1, C], F32)
        nc.sync.dma_start(out=cb_sb[:], in_=cb[:])
        sw_sb = const.tile([1, BPC * G], F32)
        nc.sync.dma_start(out=sw_sb[:], in_=sw[:])
        ones_f = const.tile([1, 128], F32)
        nc.any.memset(ones_f[:], 1.0)
        ones_bf = const.tile([1, 128], BF16)
        nc.any.memset(ones_bf[:], 1.0)

        # chunk-0 up-weights early so block-0 GEMM2 isn't starved
        wup_sb = wpool.tile([128, NKEH * DM], FP8 if G2_FP8 else BF16)
        for kk in range(4):
            nc.sync.dma_start(out=wup_sb[:, kk * DM:(kk + 1) * DM],
                              in_=wup[kk * 128:(kk + 1) * 128, :])
        upb_sb = const.tile([E, DM], F32)
        nc.sync.dma_start(out=upb_sb[:], in_=upb[:])

        # ---- feature means + routers, batch-0 first ---------------------
        # half-size [128, S/2] tiles keep the mean pool small; the router
        # matmul simply accumulates 2 partial columns per k-tile
        mf_sb = const.tile([128, BPC * KD * 2], F32)

        def batch_means(b):
            for k in range(KD):
                for h in range(2):
                    xtm = xtp.tile([128, S // 2], BF16, tag="xmean",
                                   name=f"xtm{b}{k}{h}")
                    nc.sync.dma_start(
                        out=xtm[:],
                        in_=xt[k * 128:(k + 1) * 128,
                               b * S + h * (S // 2):b * S + (h + 1) * (S // 2)])
                    col = (b * KD + k) * 2 + h
                    nc.vector.reduce_sum(mf_sb[:, col:col + 1], xtm[:],
                                         axis=AX.X)

        wsb = [None] * BPC
        bias_bf = [None] * BPC

        def batch_router(b):
            lgp = ps_s.tile([1, C], F32, tag="g1")
            for k in range(KD):
                for h in range(2):
                    col = (b * KD + k) * 2 + h
                    nc.tensor.matmul(lgp[:], lhsT=mf_sb[:, col:col + 1],
                                     rhs=cws_sb[:, k * C:(k + 1) * C],
                                     start=(k == 0 and h == 0), stop=False)
            nc.tensor.matmul(lgp[:], lhsT=ones_f[0:1, 0:1], rhs=cb_sb[:],
                             start=False, stop=True)
            lg_sb = const.tile([1, C], F32, tag=f"lg{b}")
            nc.scalar.activation(lg_sb[:], lgp[:], AF.Copy)
            nc.sync.dma_start(out=clg[b:b + 1, :], in_=lg_sb[:])

            e_sb = const.tile([1, C], F32, tag=f"esb{b}")
            nc.scalar.activation(e_sb[:], lg_sb[:], AF.Exp)
            s_sb = const.tile([1, 1], F32, tag=f"ssb{b}")
            nc.vector.reduce_sum(s_sb[:], e_sb[:], axis=AX.X)
            r_sb = const.tile([1, 1], F32, tag=f"rsb{b}")
            nc.vector.reciprocal(r_sb[:], s_sb[:])
            cwd = const.tile([1, C], F32, tag=f"cwd{b}")
            nc.vector.tensor_scalar_mul(cwd[:], e_sb[:], r_sb[0:1, 0:1])

            # flat joint weights fw[e = c*G + g] = cond[c] * stage[g]
            fw = const.tile([1, E], F32, tag=f"fw{b}")
            fw3 = fw[0:1, :].rearrange("p (c g) -> p c g", g=G)
            cwd3 = cwd[0:1, :].rearrange("p (c u) -> p c u", u=1)
            for g in range(G):
                nc.vector.tensor_scalar_mul(fw3[:, :, g:g + 1], cwd3[:],
                                            sw_sb[0:1, b * G + g:b * G + g + 1])

            # broadcast across partitions: [128, E]
            wbp = ps_s.tile([128, E], F32, tag="g1")
            nc.tensor.matmul(wbp[:], lhsT=ones_f[:], rhs=fw[:],
                             start=True, stop=True)
            ws = const.tile([128, E], F32, tag=f"ws{b}")
            nc.scalar.activation(ws[:], wbp[:], AF.Copy,
                                 scale=HS if G2_FP8 else 1.0)
            wsb[b] = ws

            # fw -> column layout [E, 1] via DVE 32x32 transpose
            tin = const.tile([32, 32], F32, tag=f"ti{b}")
            nc.any.memset(tin[:], 0.0)
            nc.vector.tensor_copy(tin[0:1, 0:E], fw[:])
            tco = const.tile([32, 32], F32, tag=f"tc{b}")
            nc.vector.transpose(tco[:], tin[:])

            # bias_term = fw @ up_b   -> [1, DM], cast to bf16
            bb = const.tile([1, DM], BF16, tag=f"bb{b}")
            for hh in range(2):
                bps = ps_s.tile([1, 512], F32, tag="g1")
                nc.tensor.matmul(bps[:], lhsT=tco[0:E, 0:1],
                                 rhs=upb_sb[0:E, hh * 512:(hh + 1) * 512],
                                 start=True, stop=True)
                nc.scalar.activation(bb[0:1, hh * 512:(hh + 1) * 512], bps[:],
                                     AF.Copy, scale=TOT if G2_FP8 else 1.0)
            bias_bf[b] = bb

        batch_means(0)
        batch_router(0)
        # remaining up-weights trickle in behind block 0's first chunks
        for kk in range(4, NKEH):
            nc.sync.dma_start(out=wup_sb[:, kk * DM:(kk + 1) * DM],
                              in_=wup[kk * 128:(kk + 1) * 128, :])
        xrb0 = xrp.tile([128, 2 * DM], F32, name="xrb0")
        for tt in range(2):
            nc.sync.dma_start(out=xrb0[:, tt * DM:(tt + 1) * DM],
                              in_=xres[tt * 128:(tt + 1) * 128, :])

        # ---- main pipeline ----------------------------------------------
        for b in range(BPC):
            for blk in range(NBLK):
                if b == 0 and blk == 2:
                    # batch-1 router work slots in once block 0/1 DMAs are
                    # queued; it only needs to finish before batch 1 starts
                    batch_means(1)
                    batch_router(1)
                t0 = b * S + blk * TB
                if b == 0 and blk == 0:
                    xtb, xrb = xtb0, xrb0
                else:
                    xtb = xtp.tile([128, KD * TB], BF16, tag="xtm")
                    for k in range(KD):
                        nc.sync.dma_start(
                            out=xtb[:, k * TB:(k + 1) * TB],
                            in_=xt[k * 128:(k + 1) * 128, t0:t0 + TB])
                    xrb = xrp.tile([128, 2 * DM], F32)
                    for tt in range(2):
                        nc.sync.dma_start(
                            out=xrb[:, tt * DM:(tt + 1) * DM],
                            in_=xres[t0 + tt * 128:t0 + (tt + 1) * 128, :])

                dps = [ps_d.tile([128, 512], F32, tag=f"d{i}", name=f"dps{i}")
                       for i in range(4)]

                for c in range(NCH):
                    htc = htp.tile([128, 4 * TB], FP8 if G2_FP8 else BF16)
                    g1s = [None] * 4
                    if b == 0 and blk == 0 and c == 0:
                        # very first chunk: k-outer order lets matmuls start as
                        # soon as each wdn k-tile's DMA lands
                        for m in range(4):
                            g1s[m] = ps_g1.tile([128, TB], F32, tag="g1",
                                                name=f"g1w{m}")
                        for k in range(KD):
                            for m in range(4):
                                col0 = c * CHW + m * 128
                                nc.tensor.matmul(
                                    g1s[m][:],
                                    lhsT=wdn_sb[:, k * EH + col0:
                                                k * EH + col0 + 128],
                                    rhs=xtb[:, k * TB:(k + 1) * TB],
                                    start=(k == 0), stop=(k == KD - 1))
                    for m in range(4):
                        if g1s[m] is not None:
                            g1 = g1s[m]
                        else:
                            g1 = ps_g1.tile([128, TB], F32, tag="g1")
                            col0 = c * CHW + m * 128
                            for k in range(KD):
                                nc.tensor.matmul(
                                    g1[:],
                                    lhsT=wdn_sb[:, k * EH + col0:
                                                k * EH + col0 + 128],
                                    rhs=xtb[:, k * TB:(k + 1) * TB],
                                    start=(k == 0), stop=(k == KD - 1))
                        eh_t = c * 4 + m
                        hslice = htc[:, m * TB:(m + 1) * TB]
                        if G2_FP8:
                            htmp = htp.tile([128, TB], BF16, tag="htmp")
                            nc.scalar.activation(htmp[:], g1[:], AF.Gelu,
                                                 bias=dbt_sb[:, eh_t:eh_t + 1])
                            hsrc = htmp[:]
                        else:
                            nc.scalar.activation(hslice, g1[:], AF.Gelu,
                                                 bias=dbt_sb[:, eh_t:eh_t + 1])
                            hsrc = hslice
                        e_idx = (c * CHW + m * 128) // HID
                        nc.vector.tensor_scalar_mul(
                            hslice, hsrc, wsb[b][:, e_idx:e_idx + 1])
                    if G2_FP8:
                        ht3 = htc[:, :].rearrange("p (m t) -> p m t", m=4)
                        wu3 = wup_sb[:, :].rearrange("p (kk dd) -> p kk dd",
                                                     kk=NKEH)
                        for tt in range(2):
                            for m in (0, 2):
                                kk = c * 4 + m
                                for hh in range(2):
                                    nc.tensor.matmul(
                                        dps[tt * 2 + hh][:],
                                        lhsT=ht3[:, m:m + 2,
                                                 tt * 128:(tt + 1) * 128],
                                        rhs=wu3[:, kk:kk + 2,
                                                hh * 512:(hh + 1) * 512],
                                        start=(c == 0 and m == 0),
                                        stop=False,
                                        perf_mode=mybir.MatmulPerfMode.DoubleRow)
                    else:
                        for tt in range(2):
                            for m in range(4):
                                lhs = htc[:, m * TB + tt * 128:
                                          m * TB + (tt + 1) * 128]
                                kk = c * 4 + m
                                for hh in range(2):
                                    nc.tensor.matmul(
                                        dps[tt * 2 + hh][:], lhsT=lhs,
                                        rhs=wup_sb[:, kk * DM + hh * 512:
                                                   kk * DM + (hh + 1) * 512],
                                        start=(c == 0 and m == 0),
                                        stop=False)

                # up_b bias term closes each accumulation group (keeps block 0
                # off the router's critical path)
                for tt in range(2):
                    for hh in range(2):
                        nc.tensor.matmul(
                            dps[tt * 2 + hh][:], lhsT=ones_bf[:],
                            rhs=bias_bf[b][0:1, hh * 512:(hh + 1) * 512],
                            start=False, stop=True)

                outb = outp.tile([128, 2 * DM], F32)
                for tt in range(2):
                    for hh in range(2):
                        sl = slice(tt * DM + hh * 512, tt * DM + (hh + 1) * 512)
                        if G2_FP8:
                            # xres comes in pre-multiplied by TOT; fold the
                            # 1/TOT rescale into the ACT eviction
                            nc.vector.tensor_tensor(
                                out=dps[tt * 2 + hh][:], in0=dps[tt * 2 + hh][:],
                                in1=xrb[:, sl], op=ALU.add)
                            nc.scalar.activation(outb[:, sl],
                                                 dps[tt * 2 + hh][:],
                                                 AF.Copy, scale=1.0 / TOT)
                        else:
                            nc.vector.tensor_tensor(
                                out=outb[:, sl], in0=dps[tt * 2 + hh][:],
                                in1=xrb[:, sl], op=ALU.add)
                for tt in range(2):
                    nc.sync.dma_start(
                        out=out[t0 + tt * 128:t0 + (tt + 1) * 128, :],
                        in_=outb[:, tt * DM:(tt + 1) * DM])

    nc.compile()
    return nc


def _get_nc():
    if "nc" not in _CACHE:
        _CACHE["nc"] = _build()
    return _CACHE["nc"]


def _softmax32(x):
    x = np.asarray(x, np.float32)
    m = x.max(axis=-1, keepdims=True)
    e = np.exp(x - m)
    return e / e.sum(axis=-1, keepdims=True)


def kernel(**inputs):
    global LAST_RESULT
    features = np.ascontiguousarray(np.asarray(inputs["features"], np.float32))
    x_raw = np.asarray(inputs["x_raw"], np.float32)
    down_w = np.asarray(inputs["down_w"], np.float32)
    down_b = np.asarray(inputs["down_b"], np.float32)
    up_w = np.asarray(inputs["up_w"], np.float32)
    up_b = np.asarray(inputs["up_b"], np.float32)
    cond_w = np.asarray(inputs["cond_w"], np.float32)
    cond_b = np.asarray(inputs["cond_b"], np.float32)
    stage_w = np.asarray(inputs["stage_w"], np.float32)
    stage_b = np.asarray(inputs["stage_b"], np.float32)

    # host-side router for the tiny stage branch (16x2048x14 input)
    stage_logits = x_raw.mean(axis=1, dtype=np.float32) @ stage_w + stage_b
    stage_weights = _softmax32(stage_logits)            # [B, G]

    wdn_np = np.ascontiguousarray(
        down_w.transpose(1, 0, 2).reshape(DM, EH)).astype(BF16_NP)
    if G2_FP8:
        wup_np = (up_w.reshape(EH, DM) * np.float32(WS)).astype(FP8_NP)
    else:
        wup_np = np.ascontiguousarray(up_w.reshape(EH, DM)).astype(BF16_NP)
    dbt_np = np.ascontiguousarray(down_b.reshape(EH).reshape(NKEH, 128).T)
    upb_np = np.ascontiguousarray(up_b)                  # [E, DM]
    cws_np = np.ascontiguousarray(cond_w / np.float32(S))
    cb_np = cond_b.reshape(1, C)

    nc = _get_nc()
    in_maps = []
    for cidx in range(NCORE):
        fs = features[BPC * cidx:BPC * (cidx + 1)].reshape(TPC, DM)
        in_maps.append({
            "xt": fs.T.astype(BF16_NP),
            "xres": fs * np.float32(TOT) if G2_FP8 else fs,
            "wdn": wdn_np,
            "wup": wup_np,
            "dbt": dbt_np,
            "upb": upb_np,
            "cws": cws_np,
            "cb": cb_np,
            "sw": np.ascontiguousarray(
                stage_weights[BPC * cidx:BPC * (cidx + 1)].reshape(1, BPC * G)),
        })

    res = run_bass_kernel_spmd(nc, in_maps, list(range(NCORE)))
    LAST_RESULT = res

    output = np.concatenate(
        [res.results[c]["out"] for c in range(NCORE)], axis=0
    ).reshape(B, S, DM)
    cond_logits = np.concatenate(
        [res.results[c]["clg"] for c in range(NCORE)], axis=0)   # [B, C]
    cond_weights = _softmax32(cond_logits)

    joint = cond_weights[:, :, None] * stage_weights[:, None, :]
    flat = joint.reshape(B, E).astype(np.float32)
    expert_loads = flat.mean(axis=0, dtype=np.float32)
    lb_loss = np.float32(E * np.sum(expert_loads * expert_loads,
                                    dtype=np.float32) * np.float32(0.01))

    return (output.astype(np.float32), cond_weights.astype(np.float32),
            stage_weights.astype(np.float32), expert_loads, lb_loss)


# revision 27
# speedup vs baseline: 1.0362x; 1.0362x over previous
"""Trainium2 Bass kernel for the DSA-MoE routing module.

Strategy: data-parallel over batch. Each of the 8 NeuronCores gets 2 full
batches (4096 tokens). Expert weights are replicated, cast to bf16 on host,
and kept SBUF-resident. Per core:

  - cond router: sum features over seq via DVE reduce on the pre-transposed
    bf16 activations, then a small fp32 matmul against cond_w/2048 (+cond_b
    via a rank-1 matmul). Final softmax happens on host from the returned
    logits; an on-device softmax produces the internal routing weights.
  - stage router: x_raw is tiny (16x2048x14) -> host computes stage softmax
    and passes the per-batch weights in.
  - main pipeline per 256-token block: GEMM1 (X.T @ W_down -> H.T in
    [EH, tokens] layout, bf16, PSUM fp32), fused Gelu+down_b eviction on
    ACT, per-(batch,expert) scale on DVE, GEMM2 accumulates
    delta = Hw.T^T @ W_up directly in PSUM across all 9 EH-chunks
    (plus a rank-1 matmul adding the up_b bias term), final eviction adds
    the fp32 residual.

The full (unsharded) inputs come in; sharding/gather happens on host.
"""

import sys

sys.path.insert(0, "/opt/trn_rl_repo")

from contextlib import ExitStack

import ml_dtypes
import numpy as np

import concourse.bass as bass  # noqa: F401  (registers bass types)
import concourse.tile as tile
from concourse import bacc, mybir
from concourse.bass_utils import run_bass_kernel_spmd

BF16, F32 = mybir.dt.bfloat16, mybir.dt.float32
FP8 = mybir.dt.float8e4
AF = mybir.ActivationFunctionType
AX = mybir.AxisListType
ALU = mybir.AluOpType
BF16_NP = ml_dtypes.bfloat16
FP8_NP = ml_dtypes.float8_e4m3

# fp8 GEMM2: h is pre-scaled by HS (off the e4m3 subnormal range) and W_up by
# WS; the final eviction divides the PSUM result by HS*WS, with the residual
# and bias terms pre-multiplied to match.
G2_FP8 = False
HS = 128.0
WS = 64.0
TOT = HS * WS

B, S, DM, HID = 16, 2048, 1024, 256
C, G = 6, 3
E = C * G                      # 18 experts
EH = E * HID                   # 4608
NCORE = 8
BPC = B // NCORE               # batches per core = 2
TPC = BPC * S                  # tokens per core = 4096
TB = 256                       # token block
NBLK = S // TB                 # blocks per batch = 8
KD = DM // 128                 # 8 k-tiles over D
NCH = 9                        # EH chunks
CHW = EH // NCH                # 512 EH cols per chunk
NKEH = EH // 128               # 36 EH k-tiles

_CACHE = {}
LAST_RESULT = None


def _build():
    nc = bacc.Bacc("TRN2", target_bir_lowering=False, debug=False,
                   num_devices=NCORE)
    xt = nc.dram_tensor("xt", [DM, TPC], BF16, kind="ExternalInput").ap()
    xres = nc.dram_tensor("xres", [TPC, DM], F32, kind="ExternalInput").ap()
    wdn = nc.dram_tensor("wdn", [DM, EH], BF16, kind="ExternalInput").ap()
    wup = nc.dram_tensor("wup", [EH, DM], FP8 if G2_FP8 else BF16,
                         kind="ExternalInput").ap()
    dbt = nc.dram_tensor("dbt", [128, NKEH], F32, kind="ExternalInput").ap()
    upb = nc.dram_tensor("upb", [E, DM], F32, kind="ExternalInput").ap()
    cws = nc.dram_tensor("cws", [DM, C], F32, kind="ExternalInput").ap()
    cb = nc.dram_tensor("cb", [1, C], F32, kind="ExternalInput").ap()
    sw = nc.dram_tensor("sw", [1, BPC * G], F32, kind="ExternalInput").ap()
    out = nc.dram_tensor("out", [TPC, DM], F32, kind="ExternalOutput").ap()
    clg = nc.dram_tensor("clg", [BPC, C], F32, kind="ExternalOutput").ap()

    with tile.TileContext(nc) as tc, ExitStack() as ctx:
        const = ctx.enter_context(tc.tile_pool(name="const", bufs=1))
        wpool = ctx.enter_context(tc.tile_pool(name="wpool", bufs=1))
        xtp = ctx.enter_context(tc.tile_pool(name="xtp", bufs=2))
        xrp = ctx.enter_context(tc.tile_pool(name="xrp", bufs=1))
        htp = ctx.enter_context(tc.tile_pool(name="htp", bufs=6))
        outp = ctx.enter_context(tc.tile_pool(name="outp", bufs=1))
        ps_g1 = ctx.enter_context(tc.tile_pool(name="psg1", bufs=3, space="PSUM"))
        ps_d = ctx.enter_context(tc.tile_pool(name="psd", bufs=1, space="PSUM"))
        ps_s = ctx.enter_context(tc.tile_pool(name="pss", bufs=1, space="PSUM"))

        # DMA issue order is roughly execution order: wdn + block-0
        # activations first (gives PE its runway), then the router means,
        # then wup (trickles in under block 0's GEMM1), then the rest.
        wdn_sb = wpool.tile([128, KD * EH], BF16)       # k-tile k at [:, k*EH:]
        for k in range(KD):
            nc.sync.dma_start(out=wdn_sb[:, k * EH:(k + 1) * EH],
                              in_=wdn[k * 128:(k + 1) * 128, :])
        xtb0 = xtp.tile([128, KD * TB], BF16, tag="xtm", name="xtb0")
        for k in range(KD):
            nc.sync.dma_start(out=xtb0[:, k * TB:(k + 1) * TB],
                              in_=xt[k * 128:(k + 1) * 128, 0:TB])
        dbt_sb = const.tile([128, NKEH], F32)
        nc.sync.dma_start(out=dbt_sb[:], in_=dbt[:])
        cws_sb = const.tile([128, KD * C], F32)
        for k in range(KD):
            nc.sync.dma_start(out=cws_sb[:, k * C:(k + 1) * C],
                              in_=cws[k * 128:(k + 1) * 128, :])
        cb_sb = const.tile([1, C], F32)
        nc.sync.dma_start(out=cb_sb[:], in_=cb[:])
        sw_sb = const.tile([1, BPC * G], F32)
        nc.sync.dma_start(out=sw_sb[:], in_=sw[:])
        ones_f = const.tile([1, 128], F32)
        nc.any.memset(ones_f[:], 1.0)
        ones_bf = const.tile([1, 128], BF16)
        nc.any.memset(ones_bf[:], 1.0)

        # chunk-0 up-weights early so block-0 GEMM2 isn't starved
        wup_sb = wpool.tile([128, NKEH * DM], FP8 if G2_FP8 else BF16)
        for kk in range(4):
            nc.sync.dma_start(out=wup_sb[:, kk * DM:(kk + 1) * DM],
                              in_=wup[kk * 128:(kk + 1) * 128, :])
        upb_sb = const.tile([E, DM], F32)
        nc.sync.dma_start(out=upb_sb[:], in_=upb[:])

        # ---- feature means + routers, batch-0 first ---------------------
        # half-size [128, S/2] tiles keep the mean pool small; the router
        # matmul simply accumulates 2 partial columns per k-tile
        mf_sb = const.tile([128, BPC * KD * 2], F32)

        def batch_means(b):
            for k in range(KD):
                for h in range(2):
                    xtm = xtp.tile([128, S // 2], BF16, tag="xmean",
                                   name=f"xtm{b}{k}{h}")
                    nc.sync.dma_start(
                        out=xtm[:],
                        in_=xt[k * 128:(k + 1) * 128,
                               b * S + h * (S // 2):b * S + (h + 1) * (S // 2)])
                    col = (b * KD + k) * 2 + h
                    nc.vector.reduce_sum(mf_sb[:, col:col + 1], xtm[:],
                                         axis=AX.X)

        wsb = [None] * BPC
        bias_bf = [None] * BPC

        def batch_router(b):
            lgp = ps_s.tile([1, C], F32, tag="small")
            for k in range(KD):
                for h in range(2):
                    col = (b * KD + k) * 2 + h
                    nc.tensor.matmul(lgp[:], lhsT=mf_sb[:, col:col + 1],
                                     rhs=cws_sb[:, k * C:(k + 1) * C],
                                     start=(k == 0 and h == 0), stop=False)
            nc.tensor.matmul(lgp[:], lhsT=ones_f[0:1, 0:1], rhs=cb_sb[:],
                             start=False, stop=True)
            lg_sb = const.tile([1, C], F32, tag=f"lg{b}")
            nc.scalar.activation(lg_sb[:], lgp[:], AF.Copy)
            nc.sync.dma_start(out=clg[b:b + 1, :], in_=lg_sb[:])

            e_sb = const.tile([1, C], F32, tag=f"esb{b}")
            nc.scalar.activation(e_sb[:], lg_sb[:], AF.Exp)
            s_sb = const.tile([1, 1], F32, tag=f"ssb{b}")
            nc.vector.reduce_sum(s_sb[:], e_sb[:], axis=AX.X)
            r_sb = const.tile([1, 1], F32, tag=f"rsb{b}")
            nc.vector.reciprocal(r_sb[:], s_sb[:])
            cwd = const.tile([1, C], F32, tag=f"cwd{b}")
            nc.vector.tensor_scalar_mul(cwd[:], e_sb[:], r_sb[0:1, 0:1])

            # flat joint weights fw[e = c*G + g] = cond[c] * stage[g]
            fw = const.tile([1, E], F32, tag=f"fw{b}")
            fw3 = fw[0:1, :].rearrange("p (c g) -> p c g", g=G)
            cwd3 = cwd[0:1, :].rearrange("p (c u) -> p c u", u=1)
            for g in range(G):
                nc.vector.tensor_scalar_mul(fw3[:, :, g:g + 1], cwd3[:],
                                            sw_sb[0:1, b * G + g:b * G + g + 1])

            # broadcast across partitions: [128, E]
            wbp = ps_s.tile([128, E], F32, tag="small")
            nc.tensor.matmul(wbp[:], lhsT=ones_f[:], rhs=fw[:],
                             start=True, stop=True)
            ws = const.tile([128, E], F32, tag=f"ws{b}")
            nc.scalar.activation(ws[:], wbp[:], AF.Copy,
                                 scale=HS if G2_FP8 else 1.0)
            wsb[b] = ws

            # fw -> column layout [E, 1] via DVE 32x32 transpose
            tin = const.tile([32, 32], F32, tag=f"ti{b}")
            nc.any.memset(tin[:], 0.0)
            nc.vector.tensor_copy(tin[0:1, 0:E], fw[:])
            tco = const.tile([32, 32], F32, tag=f"tc{b}")
            nc.vector.transpose(tco[:], tin[:])

            # bias_term = fw @ up_b   -> [1, DM], cast to bf16
            bb = const.tile([1, DM], BF16, tag=f"bb{b}")
            for hh in range(2):
                bps = ps_s.tile([1, 512], F32, tag="small")
                nc.tensor.matmul(bps[:], lhsT=tco[0:E, 0:1],
                                 rhs=upb_sb[0:E, hh * 512:(hh + 1) * 512],
                                 start=True, stop=True)
                nc.scalar.activation(bb[0:1, hh * 512:(hh + 1) * 512], bps[:],
                                     AF.Copy, scale=TOT if G2_FP8 else 1.0)
            bias_bf[b] = bb

        batch_means(0)
        batch_router(0)
        # remaining up-weights trickle in behind block 0's first chunks
        for kk in range(4, NKEH):
            nc.sync.dma_start(out=wup_sb[:, kk * DM:(kk + 1) * DM],
                              in_=wup[kk * 128:(kk + 1) * 128, :])
        xrb0 = xrp.tile([128, 2 * DM], F32, name="xrb0")
        for tt in range(2):
            nc.sync.dma_start(out=xrb0[:, tt * DM:(tt + 1) * DM],
                              in_=xres[tt * 128:(tt + 1) * 128, :])

        # ---- main pipeline ----------------------------------------------
        for b in range(BPC):
            for blk in range(NBLK):
                if b == 0 and blk == 2:
                    # batch-1 means DMAs queue up behind block 0/1 loads; the
                    # router matmuls are emitted later (block 6) so the PE
                    # never waits on them in-order
                    batch_means(1)
                if b == 0 and blk == 6:
                    batch_router(1)
                t0 = b * S + blk * TB
                if b == 0 and blk == 0:
                    xtb, xrb = xtb0, xrb0
                else:
                    xtb = xtp.tile([128, KD * TB], BF16, tag="xtm")
                    for k in range(KD):
                        nc.sync.dma_start(
                            out=xtb[:, k * TB:(k + 1) * TB],
                            in_=xt[k * 128:(k + 1) * 128, t0:t0 + TB])
                    xrb = xrp.tile([128, 2 * DM], F32)
                    for tt in range(2):
                        nc.sync.dma_start(
                            out=xrb[:, tt * DM:(tt + 1) * DM],
                            in_=xres[t0 + tt * 128:t0 + (tt + 1) * 128, :])

                dps = [ps_d.tile([128, 512], F32, tag=f"d{i}", name=f"dps{i}")
                       for i in range(4)]

                for c in range(NCH):
                    htc = htp.tile([128, 4 * TB], FP8 if G2_FP8 else BF16)
                    g1s = [None] * 4
                    if b == 0 and blk == 0 and c == 0:
                        # very first chunk: k-outer order lets matmuls start as
                        # soon as each wdn k-tile's DMA lands (3 tiles = G1
                        # ring size; the 4th follows the normal path)
                        for m in range(3):
                            g1s[m] = ps_g1.tile([128, TB], F32, tag="g1",
                                                name=f"g1w{m}")
                        for k in range(KD):
                            for m in range(3):
                                col0 = c * CHW + m * 128
                                nc.tensor.matmul(
                                    g1s[m][:],
                                    lhsT=wdn_sb[:, k * EH + col0:
                                                k * EH + col0 + 128],
                                    rhs=xtb[:, k * TB:(k + 1) * TB],
                                    start=(k == 0), stop=(k == KD - 1))
                    for m in range(4):
                        if g1s[m] is not None:
                            g1 = g1s[m]
                        else:
                            g1 = ps_g1.tile([128, TB], F32, tag="g1")
                            col0 = c * CHW + m * 128
                            for k in range(KD):
                                nc.tensor.matmul(
                                    g1[:],
                                    lhsT=wdn_sb[:, k * EH + col0:
                                                k * EH + col0 + 128],
                                    rhs=xtb[:, k * TB:(k + 1) * TB],
                                    start=(k == 0), stop=(k == KD - 1))
                        eh_t = c * 4 + m
                        hslice = htc[:, m * TB:(m + 1) * TB]
                        if G2_FP8:
                            htmp = htp.tile([128, TB], BF16, tag="htmp")
                            nc.scalar.activation(htmp[:], g1[:], AF.Gelu,
                                                 bias=dbt_sb[:, eh_t:eh_t + 1])
                            hsrc = htmp[:]
                        else:
                            nc.scalar.activation(hslice, g1[:], AF.Gelu,
                                                 bias=dbt_sb[:, eh_t:eh_t + 1])
                            hsrc = hslice
                        e_idx = (c * CHW + m * 128) // HID
                        nc.vector.tensor_scalar_mul(
                            hslice, hsrc, wsb[b][:, e_idx:e_idx + 1])
                    if G2_FP8:
                        ht3 = htc[:, :].rearrange("p (m t) -> p m t", m=4)
                        wu3 = wup_sb[:, :].rearrange("p (kk dd) -> p kk dd",
                                                     kk=NKEH)
                        for tt in range(2):
                            for m in (0, 2):
                                kk = c * 4 + m
                                for hh in range(2):
                                    nc.tensor.matmul(
                                        dps[tt * 2 + hh][:],
                                        lhsT=ht3[:, m:m + 2,
                                                 tt * 128:(tt + 1) * 128],
                                        rhs=wu3[:, kk:kk + 2,
                                                hh * 512:(hh + 1) * 512],
                                        start=(c == 0 and m == 0),
                                        stop=False,
                                        perf_mode=mybir.MatmulPerfMode.DoubleRow)
                    else:
                        for tt in range(2):
                            for m in range(4):
                                lhs = htc[:, m * TB + tt * 128:
                                          m * TB + (tt + 1) * 128]
                                kk = c * 4 + m
                                for hh in range(2):
                                    nc.tensor.matmul(
                                        dps[tt * 2 + hh][:], lhsT=lhs,
                                        rhs=wup_sb[:, kk * DM + hh * 512:
                                                   kk * DM + (hh + 1) * 512],
                                        start=(c == 0 and m == 0),
                                        stop=False)

                # up_b bias term closes each accumulation group (keeps block 0
                # off the router's critical path)
                for tt in range(2):
                    for hh in range(2):
                        nc.tensor.matmul(
                            dps[tt * 2 + hh][:], lhsT=ones_bf[:],
                            rhs=bias_bf[b][0:1, hh * 512:(hh + 1) * 512],
                            start=False, stop=True)

                outb = outp.tile([128, 2 * DM], F32)
                for tt in range(2):
                    for hh in range(2):
                        sl = slice(tt * DM + hh * 512, tt * DM + (hh + 1) * 512)
                        if G2_FP8:
                            # xres comes in pre-multiplied by TOT; fold the
                            # 1/TOT rescale into the ACT eviction
                            nc.vector.tensor_tensor(
                                out=dps[tt * 2 + hh][:], in0=dps[tt * 2 + hh][:],
                                in1=xrb[:, sl], op=ALU.add)
                            nc.scalar.activation(outb[:, sl],
                                                 dps[tt * 2 + hh][:],
                                                 AF.Copy, scale=1.0 / TOT)
                        else:
                            nc.vector.tensor_tensor(
                                out=outb[:, sl], in0=dps[tt * 2 + hh][:],
                                in1=xrb[:, sl], op=ALU.add)
                for tt in range(2):
                    nc.sync.dma_start(
                        out=out[t0 + tt * 128:t0 + (tt + 1) * 128, :],
                        in_=outb[:, tt * DM:(tt + 1) * DM])

    nc.compile()
    return nc


def _get_nc():
    if "nc" not in _CACHE:
        _CACHE["nc"] = _build()
    return _CACHE["nc"]


def _softmax32(x):
    x = np.asarray(x, np.float32)
    m = x.max(axis=-1, keepdims=True)
    e = np.exp(x - m)
    return e / e.sum(axis=-1, keepdims=True)


def kernel(**inputs):
    global LAST_RESULT
    features = np.ascontiguousarray(np.asarray(inputs["features"], np.float32))
    x_raw = np.asarray(inputs["x_raw"], np.float32)
    down_w = np.asarray(inputs["down_w"], np.float32)
    down_b = np.asarray(inputs["down_b"], np.float32)
    up_w = np.asarray(inputs["up_w"], np.float32)
    up_b = np.asarray(inputs["up_b"], np.float32)
    cond_w = np.asarray(inputs["cond_w"], np.float32)
    cond_b = np.asarray(inputs["cond_b"], np.float32)
    stage_w = np.asarray(inputs["stage_w"], np.float32)
    stage_b = np.asarray(inputs["stage_b"], np.float32)

    # host-side router for the tiny stage branch (16x2048x14 input)
    stage_logits = x_raw.mean(axis=1, dtype=np.float32) @ stage_w + stage_b
    stage_weights = _softmax32(stage_logits)            # [B, G]

    wdn_np = np.ascontiguousarray(
        down_w.transpose(1, 0, 2).reshape(DM, EH)).astype(BF16_NP)
    if G2_FP8:
        wup_np = (up_w.reshape(EH, DM) * np.float32(WS)).astype(FP8_NP)
    else:
        wup_np = np.ascontiguousarray(up_w.reshape(EH, DM)).astype(BF16_NP)
    dbt_np = np.ascontiguousarray(down_b.reshape(EH).reshape(NKEH, 128).T)
    upb_np = np.ascontiguousarray(up_b)                  # [E, DM]
    cws_np = np.ascontiguousarray(cond_w / np.float32(S))
    cb_np = cond_b.reshape(1, C)

    nc = _get_nc()
    in_maps = []
    for cidx in range(NCORE):
        fs = features[BPC * cidx:BPC * (cidx + 1)].reshape(TPC, DM)
        in_maps.append({
            "xt": fs.T.astype(BF16_NP),
            "xres": fs * np.float32(TOT) if G2_FP8 else fs,
            "wdn": wdn_np,
            "wup": wup_np,
            "dbt": dbt_np,
            "upb": upb_np,
            "cws": cws_np,
            "cb": cb_np,
            "sw": np.ascontiguousarray(
                stage_weights[BPC * cidx:BPC * (cidx + 1)].reshape(1, BPC * G)),
        })

    res = run_bass_kernel_spmd(nc, in_maps, list(range(NCORE)))
    LAST_RESULT = res

    output = np.concatenate(
        [res.results[c]["out"] for c in range(NCORE)], axis=0
    ).reshape(B, S, DM)
    cond_logits = np.concatenate(
        [res.results[c]["clg"] for c in range(NCORE)], axis=0)   # [B, C]
    cond_weights = _softmax32(cond_logits)

    joint = cond_weights[:, :, None] * stage_weights[:, None, :]
    flat = joint.reshape(B, E).astype(np.float32)
    expert_loads = flat.mean(axis=0, dtype=np.float32)
    lb_loss = np.float32(E * np.sum(expert_loads * expert_loads,
                                    dtype=np.float32) * np.float32(0.01))

    return (output.astype(np.float32), cond_weights.astype(np.float32),
            stage_weights.astype(np.float32), expert_loads, lb_loss)


# revision 31
# speedup vs baseline: 1.0399x; 1.0036x over previous
"""Trainium2 Bass kernel for the DSA-MoE routing module.

Strategy: data-parallel over batch. Each of the 8 NeuronCores gets 2 full
batches (4096 tokens). Expert weights are replicated, cast to bf16 on host,
and kept SBUF-resident. Per core:

  - cond router: sum features over seq via DVE reduce on the pre-transposed
    bf16 activations, then a small fp32 matmul against cond_w/2048 (+cond_b
    via a rank-1 matmul). Final softmax happens on host from the returned
    logits; an on-device softmax produces the internal routing weights.
  - stage router: x_raw is tiny (16x2048x14) -> host computes stage softmax
    and passes the per-batch weights in.
  - main pipeline per 256-token block: GEMM1 (X.T @ W_down -> H.T in
    [EH, tokens] layout, bf16, PSUM fp32), fused Gelu+down_b eviction on
    ACT, per-(batch,expert) scale on DVE, GEMM2 accumulates
    delta = Hw.T^T @ W_up directly in PSUM across all 9 EH-chunks
    (plus a rank-1 matmul adding the up_b bias term), final eviction adds
    the fp32 residual.

The full (unsharded) inputs come in; sharding/gather happens on host.
"""

import sys

sys.path.insert(0, "/opt/trn_rl_repo")

from contextlib import ExitStack

import ml_dtypes
import numpy as np

import concourse.bass as bass  # noqa: F401  (registers bass types)
import concourse.tile as tile
from concourse import bacc, mybir
from concourse.bass_utils import run_bass_kernel_spmd

BF16, F32 = mybir.dt.bfloat16, mybir.dt.float32
FP8 = mybir.dt.float8e4
AF = mybir.ActivationFunctionType
AX = mybir.AxisListType
ALU = mybir.AluOpType
BF16_NP = ml_dtypes.bfloat16
FP8_NP = ml_dtypes.float8_e4m3

# fp8 GEMM2: h is pre-scaled by HS (off the e4m3 subnormal range) and W_up by
# WS; the final eviction divides the PSUM result by HS*WS, with the residual
# and bias terms pre-multiplied to match.
G2_FP8 = False
HS = 128.0
WS = 64.0
TOT = HS * WS

B, S, DM, HID = 16, 2048, 1024, 256
C, G = 6, 3
E = C * G                      # 18 experts
EH = E * HID                   # 4608
NCORE = 8
BPC = B // NCORE               # batches per core = 2
TPC = BPC * S                  # tokens per core = 4096
TB = 256                       # token block
NBLK = S // TB                 # blocks per batch = 8
KD = DM // 128                 # 8 k-tiles over D
NCH = 9                        # EH chunks
CHW = EH // NCH                # 512 EH cols per chunk
NKEH = EH // 128               # 36 EH k-tiles

_CACHE = {}
LAST_RESULT = None


def _build():
    nc = bacc.Bacc("TRN2", target_bir_lowering=False, debug=False,
                   num_devices=NCORE)
    xt = nc.dram_tensor("xt", [DM, TPC], BF16, kind="ExternalInput").ap()
    xres = nc.dram_tensor("xres", [TPC, DM], F32, kind="ExternalInput").ap()
    wdn = nc.dram_tensor("wdn", [DM, EH], BF16, kind="ExternalInput").ap()
    wup = nc.dram_tensor("wup", [EH, DM], FP8 if G2_FP8 else BF16,
                         kind="ExternalInput").ap()
    dbt = nc.dram_tensor("dbt", [128, NKEH], F32, kind="ExternalInput").ap()
    upb = nc.dram_tensor("upb", [E, DM], F32, kind="ExternalInput").ap()
    cws = nc.dram_tensor("cws", [DM, C], F32, kind="ExternalInput").ap()
    cb = nc.dram_tensor("cb", [1, C], F32, kind="ExternalInput").ap()
    sw = nc.dram_tensor("sw", [1, BPC * G], F32, kind="ExternalInput").ap()
    out = nc.dram_tensor("out", [TPC, DM], F32, kind="ExternalOutput").ap()
    clg = nc.dram_tensor("clg", [BPC, C], F32, kind="ExternalOutput").ap()

    with tile.TileContext(nc) as tc, ExitStack() as ctx:
        const = ctx.enter_context(tc.tile_pool(name="const", bufs=1))
        wpool = ctx.enter_context(tc.tile_pool(name="wpool", bufs=1))
        xtp = ctx.enter_context(tc.tile_pool(name="xtp", bufs=2))
        xrp = ctx.enter_context(tc.tile_pool(name="xrp", bufs=1))
        htp = ctx.enter_context(tc.tile_pool(name="htp", bufs=1))
        outp = ctx.enter_context(tc.tile_pool(name="outp", bufs=1))
        ps_g1 = ctx.enter_context(tc.tile_pool(name="psg1", bufs=3, space="PSUM"))
        ps_d = ctx.enter_context(tc.tile_pool(name="psd", bufs=1, space="PSUM"))
        ps_s = ctx.enter_context(tc.tile_pool(name="pss", bufs=1, space="PSUM"))

        # DMA issue order is roughly execution order: wdn + block-0
        # activations first (gives PE its runway), then the router means,
        # then wup (trickles in under block 0's GEMM1), then the rest.
        wdn_sb = wpool.tile([128, KD * EH], BF16)       # k-tile k at [:, k*EH:]
        xtb0 = xtp.tile([128, KD * TB], BF16, tag="xtm", name="xtb0")
        for k in range(KD):
            nc.sync.dma_start(out=wdn_sb[:, k * EH:(k + 1) * EH],
                              in_=wdn[k * 128:(k + 1) * 128, :])
            nc.sync.dma_start(out=xtb0[:, k * TB:(k + 1) * TB],
                              in_=xt[k * 128:(k + 1) * 128, 0:TB])
        dbt_sb = const.tile([128, NKEH], F32)
        nc.sync.dma_start(out=dbt_sb[:], in_=dbt[:])
        cws_sb = const.tile([128, KD * C], F32)
        for k in range(KD):
            nc.sync.dma_start(out=cws_sb[:, k * C:(k + 1) * C],
                              in_=cws[k * 128:(k + 1) * 128, :])
        cb_sb = const.tile([1, C], F32)
        nc.sync.dma_start(out=cb_sb[:], in_=cb[:])
        sw_sb = const.tile([1, BPC * G], F32)
        nc.sync.dma_start(out=sw_sb[:], in_=sw[:])
        ones_f = const.tile([1, 128], F32)
        nc.any.memset(ones_f[:], 1.0)
        ones_bf = const.tile([1, 128], BF16)
        nc.any.memset(ones_bf[:], 1.0)

        # chunk-0 up-weights early so block-0 GEMM2 isn't starved
        wup_sb = wpool.tile([128, NKEH * DM], FP8 if G2_FP8 else BF16)
        for kk in range(4):
            nc.sync.dma_start(out=wup_sb[:, kk * DM:(kk + 1) * DM],
                              in_=wup[kk * 128:(kk + 1) * 128, :])
        upb_sb = const.tile([E, DM], F32)
        nc.sync.dma_start(out=upb_sb[:], in_=upb[:])

        # ---- feature means + routers, batch-0 first ---------------------
        # half-size [128, S/2] tiles keep the mean pool small; the router
        # matmul simply accumulates 2 partial columns per k-tile
        mf_sb = const.tile([128, BPC * KD * 2], F32)

        def batch_means(b):
            for k in range(KD):
                for h in range(2):
                    xtm = xtp.tile([128, S // 2], BF16, tag="xmean",
                                   name=f"xtm{b}{k}{h}")
                    nc.sync.dma_start(
                        out=xtm[:],
                        in_=xt[k * 128:(k + 1) * 128,
                               b * S + h * (S // 2):b * S + (h + 1) * (S // 2)])
                    col = (b * KD + k) * 2 + h
                    nc.vector.reduce_sum(mf_sb[:, col:col + 1], xtm[:],
                                         axis=AX.X)

        wsb = [None] * BPC
        bias_bf = [None] * BPC

        def batch_router(b):
            lgp = ps_s.tile([1, C], F32, tag="small")
            for k in range(KD):
                for h in range(2):
                    col = (b * KD + k) * 2 + h
                    nc.tensor.matmul(lgp[:], lhsT=mf_sb[:, col:col + 1],
                                     rhs=cws_sb[:, k * C:(k + 1) * C],
                                     start=(k == 0 and h == 0), stop=False)
            nc.tensor.matmul(lgp[:], lhsT=ones_f[0:1, 0:1], rhs=cb_sb[:],
                             start=False, stop=True)
            lg_sb = const.tile([1, C], F32, tag=f"lg{b}")
            nc.scalar.activation(lg_sb[:], lgp[:], AF.Copy)
            nc.sync.dma_start(out=clg[b:b + 1, :], in_=lg_sb[:])

            e_sb = const.tile([1, C], F32, tag=f"esb{b}")
            nc.scalar.activation(e_sb[:], lg_sb[:], AF.Exp)
            s_sb = const.tile([1, 1], F32, tag=f"ssb{b}")
            nc.vector.reduce_sum(s_sb[:], e_sb[:], axis=AX.X)
            r_sb = const.tile([1, 1], F32, tag=f"rsb{b}")
            nc.vector.reciprocal(r_sb[:], s_sb[:])
            cwd = const.tile([1, C], F32, tag=f"cwd{b}")
            nc.vector.tensor_scalar_mul(cwd[:], e_sb[:], r_sb[0:1, 0:1])

            # flat joint weights fw[e = c*G + g] = cond[c] * stage[g]
            fw = const.tile([1, E], F32, tag=f"fw{b}")
            fw3 = fw[0:1, :].rearrange("p (c g) -> p c g", g=G)
            cwd3 = cwd[0:1, :].rearrange("p (c u) -> p c u", u=1)
            for g in range(G):
                nc.vector.tensor_scalar_mul(fw3[:, :, g:g + 1], cwd3[:],
                                            sw_sb[0:1, b * G + g:b * G + g + 1])

            # broadcast across partitions: [128, E]
            wbp = ps_s.tile([128, E], F32, tag="small")
            nc.tensor.matmul(wbp[:], lhsT=ones_f[:], rhs=fw[:],
                             start=True, stop=True)
            ws = const.tile([128, E], F32, tag=f"ws{b}")
            nc.scalar.activation(ws[:], wbp[:], AF.Copy,
                                 scale=HS if G2_FP8 else 1.0)
            wsb[b] = ws

            # fw -> column layout [E, 1] via DVE 32x32 transpose
            tin = const.tile([32, 32], F32, tag=f"ti{b}")
            nc.any.memset(tin[:], 0.0)
            nc.vector.tensor_copy(tin[0:1, 0:E], fw[:])
            tco = const.tile([32, 32], F32, tag=f"tc{b}")
            nc.vector.transpose(tco[:], tin[:])

            # bias_term = fw @ up_b   -> [1, DM], cast to bf16
            bb = const.tile([1, DM], BF16, tag=f"bb{b}")
            for hh in range(2):
                bps = ps_s.tile([1, 512], F32, tag="small")
                nc.tensor.matmul(bps[:], lhsT=tco[0:E, 0:1],
                                 rhs=upb_sb[0:E, hh * 512:(hh + 1) * 512],
                                 start=True, stop=True)
                nc.scalar.activation(bb[0:1, hh * 512:(hh + 1) * 512], bps[:],
                                     AF.Copy, scale=TOT if G2_FP8 else 1.0)
            bias_bf[b] = bb

        batch_means(0)
        # remaining up-weights trickle in behind block 0's GEMM1 prologue
        for kk in range(4, NKEH):
            nc.sync.dma_start(out=wup_sb[:, kk * DM:(kk + 1) * DM],
                              in_=wup[kk * 128:(kk + 1) * 128, :])
        xrb0 = xrp.tile([128, 2 * DM], F32, name="xrb0", tag="xrb")
        for tt in range(2):
            nc.sync.dma_start(out=xrb0[:, tt * DM:(tt + 1) * DM],
                              in_=xres[tt * 128:(tt + 1) * 128, :])

        # ---- main pipeline ----------------------------------------------
        def emit_g1(xtb, htc, c):
            """GEMM1 for one EH chunk + fused gelu eviction (unscaled)."""
            for m in range(4):
                g1 = ps_g1.tile([128, TB], F32, tag="g1", name="g1")
                col0 = c * CHW + m * 128
                for k in range(KD):
                    nc.tensor.matmul(
                        g1[:],
                        lhsT=wdn_sb[:, k * EH + col0:k * EH + col0 + 128],
                        rhs=xtb[:, k * TB:(k + 1) * TB],
                        start=(k == 0), stop=(k == KD - 1))
                eh_t = c * 4 + m
                nc.scalar.activation(htc[:, m * TB:(m + 1) * TB], g1[:],
                                     AF.Gelu, bias=dbt_sb[:, eh_t:eh_t + 1])

        def emit_scale(htc, hdst, c, b):
            """Per-(batch, expert) routing-weight scale (+fp8 cast)."""
            for m in range(4):
                e_idx = (c * CHW + m * 128) // HID
                nc.vector.tensor_scalar_mul(
                    hdst[:, m * TB:(m + 1) * TB], htc[:, m * TB:(m + 1) * TB],
                    wsb[b][:, e_idx:e_idx + 1])

        def emit_g2(htc, dps, c):
            if G2_FP8:
                ht3 = htc[:, :].rearrange("p (m t) -> p m t", m=4)
                wu3 = wup_sb[:, :].rearrange("p (kk dd) -> p kk dd", kk=NKEH)
                for tt in range(2):
                    for m in (0, 2):
                        kk = c * 4 + m
                        for hh in range(2):
                            nc.tensor.matmul(
                                dps[tt * 2 + hh][:],
                                lhsT=ht3[:, m:m + 2, tt * 128:(tt + 1) * 128],
                                rhs=wu3[:, kk:kk + 2, hh * 512:(hh + 1) * 512],
                                start=(c == 0 and m == 0), stop=False,
                                perf_mode=mybir.MatmulPerfMode.DoubleRow)
            else:
                for tt in range(2):
                    for m in range(4):
                        lhs = htc[:, m * TB + tt * 128:m * TB + (tt + 1) * 128]
                        kk = c * 4 + m
                        for hh in range(2):
                            nc.tensor.matmul(
                                dps[tt * 2 + hh][:], lhsT=lhs,
                                rhs=wup_sb[:, kk * DM + hh * 512:
                                           kk * DM + (hh + 1) * 512],
                                start=(c == 0 and m == 0), stop=False)

        def emit_tail(dps, xrb, t0, b):
            # up_b bias term closes each accumulation group; eviction adds
            # the fp32 residual
            outb = outp.tile([128, 2 * DM], F32, name="outb")
            for tt in range(2):
                for hh in range(2):
                    nc.tensor.matmul(
                        dps[tt * 2 + hh][:], lhsT=ones_bf[:],
                        rhs=bias_bf[b][0:1, hh * 512:(hh + 1) * 512],
                        start=False, stop=True)
                    sl = slice(tt * DM + hh * 512, tt * DM + (hh + 1) * 512)
                    if G2_FP8:
                        nc.vector.tensor_tensor(
                            out=dps[tt * 2 + hh][:], in0=dps[tt * 2 + hh][:],
                            in1=xrb[:, sl], op=ALU.add)
                        nc.scalar.activation(outb[:, sl], dps[tt * 2 + hh][:],
                                             AF.Copy, scale=1.0 / TOT)
                    else:
                        nc.vector.tensor_tensor(
                            out=outb[:, sl], in0=dps[tt * 2 + hh][:],
                            in1=xrb[:, sl], op=ALU.add)
            for tt in range(2):
                nc.sync.dma_start(
                    out=out[t0 + tt * 128:t0 + (tt + 1) * 128, :],
                    in_=outb[:, tt * DM:(tt + 1) * DM])

        HDT = FP8 if G2_FP8 else BF16

        # block (0,0) prologue: all of GEMM1 runs before the router in PE
        # order, so the PE never waits on the means DMAs; scale + GEMM2
        # follow once the router lands.
        dps = [ps_d.tile([128, 512], F32, tag=f"d{i}", name=f"dps{i}")
               for i in range(4)]
        htcs0 = [htp.tile([128, 4 * TB], HDT, name=f"htc0c{c}",
                          tag=f"h{c}", bufs=1)
                 for c in range(NCH)]
        for c in range(NCH):
            emit_g1(xtb0, htcs0[c], c)
        batch_router(0)
        for c in range(NCH):
            emit_scale(htcs0[c], htcs0[c], c, 0)
            emit_g2(htcs0[c], dps, c)
        emit_tail(dps, xrb0, 0, 0)

        for b in range(BPC):
            for blk in range(NBLK):
                if b == 0 and blk == 0:
                    continue
                if b == 0 and blk == 2:
                    # batch-1 means DMAs queue behind block 0/1 loads; its
                    # router matmuls are emitted at block 6 so the PE never
                    # waits on them in-order
                    batch_means(1)
                if b == 0 and blk == 6:
                    batch_router(1)
                t0 = b * S + blk * TB
                xtb = xtp.tile([128, KD * TB], BF16, tag="xtm", name="xtb")
                for k in range(KD):
                    nc.sync.dma_start(
                        out=xtb[:, k * TB:(k + 1) * TB],
                        in_=xt[k * 128:(k + 1) * 128, t0:t0 + TB])
                xrb = xrp.tile([128, 2 * DM], F32, name="xrb", tag="xrb")
                for tt in range(2):
                    nc.sync.dma_start(
                        out=xrb[:, tt * DM:(tt + 1) * DM],
                        in_=xres[t0 + tt * 128:t0 + (tt + 1) * 128, :])

                dps = [ps_d.tile([128, 512], F32, tag=f"d{i}", name=f"dps{i}")
                       for i in range(4)]
                for c in range(NCH):
                    htc = htp.tile([128, 4 * TB], HDT, name="htc",
                                   tag=f"h{c}", bufs=1)
                    emit_g1(xtb, htc, c)
                    emit_scale(htc, htc, c, b)
                    emit_g2(htc, dps, c)
                emit_tail(dps, xrb, t0, b)

    nc.compile()
    return nc


def _get_nc():
    if "nc" not in _CACHE:
        _CACHE["nc"] = _build()
    return _CACHE["nc"]


def _softmax32(x):
    x = np.asarray(x, np.float32)
    m = x.max(axis=-1, keepdims=True)
    e = np.exp(x - m)
    return e / e.sum(axis=-1, keepdims=True)


def kernel(**inputs):
    global LAST_RESULT
    features = np.ascontiguousarray(np.asarray(inputs["features"], np.float32))
    x_raw = np.asarray(inputs["x_raw"], np.float32)
    down_w = np.asarray(inputs["down_w"], np.float32)
    down_b = np.asarray(inputs["down_b"], np.float32)
    up_w = np.asarray(inputs["up_w"], np.float32)
    up_b = np.asarray(inputs["up_b"], np.float32)
    cond_w = np.asarray(inputs["cond_w"], np.float32)
    cond_b = np.asarray(inputs["cond_b"], np.float32)
    stage_w = np.asarray(inputs["stage_w"], np.float32)
    stage_b = np.asarray(inputs["stage_b"], np.float32)

    # host-side router for the tiny stage branch (16x2048x14 input)
    stage_logits = x_raw.mean(axis=1, dtype=np.float32) @ stage_w + stage_b
    stage_weights = _softmax32(stage_logits)            # [B, G]

    wdn_np = np.ascontiguousarray(
        down_w.transpose(1, 0, 2).reshape(DM, EH)).astype(BF16_NP)
    if G2_FP8:
        wup_np = (up_w.reshape(EH, DM) * np.float32(WS)).astype(FP8_NP)
    else:
        wup_np = np.ascontiguousarray(up_w.reshape(EH, DM)).astype(BF16_NP)
    dbt_np = np.ascontiguousarray(down_b.reshape(EH).reshape(NKEH, 128).T)
    upb_np = np.ascontiguousarray(up_b)                  # [E, DM]
    cws_np = np.ascontiguousarray(cond_w / np.float32(S))
    cb_np = cond_b.reshape(1, C)

    nc = _get_nc()
    in_maps = []
    for cidx in range(NCORE):
        fs = features[BPC * cidx:BPC * (cidx + 1)].reshape(TPC, DM)
        in_maps.append({
            "xt": fs.T.astype(BF16_NP),
            "xres": fs * np.float32(TOT) if G2_FP8 else fs,
            "wdn": wdn_np,
            "wup": wup_np,
            "dbt": dbt_np,
            "upb": upb_np,
            "cws": cws_np,
            "cb": cb_np,
            "sw": np.ascontiguousarray(
                stage_weights[BPC * cidx:BPC * (cidx + 1)].reshape(1, BPC * G)),
        })

    res = run_bass_kernel_spmd(nc, in_maps, list(range(NCORE)))
    LAST_RESULT = res

    output = np.concatenate(
        [res.results[c]["out"] for c in range(NCORE)], axis=0
    ).reshape(B, S, DM)
    cond_logits = np.concatenate(
        [res.results[c]["clg"] for c in range(NCORE)], axis=0)   # [B, C]
    cond_weights = _softmax32(cond_logits)

    joint = cond_weights[:, :, None] * stage_weights[:, None, :]
    flat = joint.reshape(B, E).astype(np.float32)
    expert_loads = flat.mean(axis=0, dtype=np.float32)
    lb_loss = np.float32(E * np.sum(expert_loads * expert_loads,
                                    dtype=np.float32) * np.float32(0.01))

    return (output.astype(np.float32), cond_weights.astype(np.float32),
            stage_weights.astype(np.float32), expert_loads, lb_loss)


# revision 34
# speedup vs baseline: 1.0734x; 1.0322x over previous
"""Trainium2 Bass kernel for the DSA-MoE routing module.

Strategy: data-parallel over batch. Each of the 8 NeuronCores gets 2 full
batches (4096 tokens). Expert weights are replicated, cast to bf16 on host,
and kept SBUF-resident. Per core:

  - cond router: sum features over seq via DVE reduce on the pre-transposed
    bf16 activations, then a small fp32 matmul against cond_w/2048 (+cond_b
    via a rank-1 matmul). Final softmax happens on host from the returned
    logits; an on-device softmax produces the internal routing weights.
  - stage router: x_raw is tiny (16x2048x14) -> host computes stage softmax
    and passes the per-batch weights in.
  - main pipeline per 256-token block: GEMM1 (X.T @ W_down -> H.T in
    [EH, tokens] layout, bf16, PSUM fp32), fused Gelu+down_b eviction on
    ACT, per-(batch,expert) scale on DVE, GEMM2 accumulates
    delta = Hw.T^T @ W_up directly in PSUM across all 9 EH-chunks
    (plus a rank-1 matmul adding the up_b bias term), final eviction adds
    the fp32 residual.

The full (unsharded) inputs come in; sharding/gather happens on host.
"""

import sys

sys.path.insert(0, "/opt/trn_rl_repo")

from contextlib import ExitStack

import ml_dtypes
import numpy as np

import concourse.bass as bass  # noqa: F401  (registers bass types)
import concourse.tile as tile
from concourse import bacc, mybir
from concourse.bass_utils import run_bass_kernel_spmd

BF16, F32 = mybir.dt.bfloat16, mybir.dt.float32
FP8 = mybir.dt.float8e4
AF = mybir.ActivationFunctionType
AX = mybir.AxisListType
ALU = mybir.AluOpType
BF16_NP = ml_dtypes.bfloat16
FP8_NP = ml_dtypes.float8_e4m3

# fp8 GEMM2: h is pre-scaled by HS (off the e4m3 subnormal range) and W_up by
# WS; the final eviction divides the PSUM result by HS*WS, with the residual
# and bias terms pre-multiplied to match.
G2_FP8 = False
HS = 128.0
WS = 64.0
TOT = HS * WS

B, S, DM, HID = 16, 2048, 1024, 256
C, G = 6, 3
E = C * G                      # 18 experts
EH = E * HID                   # 4608
NCORE = 8
BPC = B // NCORE               # batches per core = 2
TPC = BPC * S                  # tokens per core = 4096
TB = 256                       # token block
NBLK = S // TB                 # blocks per batch = 8
KD = DM // 128                 # 8 k-tiles over D
NCH = 9                        # EH chunks
CHW = EH // NCH                # 512 EH cols per chunk
NKEH = EH // 128               # 36 EH k-tiles

_CACHE = {}
LAST_RESULT = None


def _build():
    nc = bacc.Bacc("TRN2", target_bir_lowering=False, debug=False,
                   num_devices=NCORE)
    xt = nc.dram_tensor("xt", [DM, TPC], BF16, kind="ExternalInput").ap()
    xres = nc.dram_tensor("xres", [TPC, DM], F32, kind="ExternalInput").ap()
    wdn = nc.dram_tensor("wdn", [DM, EH], BF16, kind="ExternalInput").ap()
    wup = nc.dram_tensor("wup", [EH, DM], FP8 if G2_FP8 else BF16,
                         kind="ExternalInput").ap()
    dbt = nc.dram_tensor("dbt", [128, NKEH], F32, kind="ExternalInput").ap()
    upb = nc.dram_tensor("upb", [E, DM], F32, kind="ExternalInput").ap()
    cws = nc.dram_tensor("cws", [DM, C], F32, kind="ExternalInput").ap()
    cb = nc.dram_tensor("cb", [1, C], F32, kind="ExternalInput").ap()
    sw = nc.dram_tensor("sw", [1, BPC * G], F32, kind="ExternalInput").ap()
    fw0 = nc.dram_tensor("fw0", [1, E], F32, kind="ExternalInput").ap()
    out = nc.dram_tensor("out", [TPC, DM], F32, kind="ExternalOutput").ap()
    clg = nc.dram_tensor("clg", [BPC, C], F32, kind="ExternalOutput").ap()

    with tile.TileContext(nc) as tc, ExitStack() as ctx:
        const = ctx.enter_context(tc.tile_pool(name="const", bufs=1))
        wpool = ctx.enter_context(tc.tile_pool(name="wpool", bufs=1))
        xtp = ctx.enter_context(tc.tile_pool(name="xtp", bufs=2))
        xrp = ctx.enter_context(tc.tile_pool(name="xrp", bufs=1))
        htp = ctx.enter_context(tc.tile_pool(name="htp", bufs=1))
        outp = ctx.enter_context(tc.tile_pool(name="outp", bufs=1))
        ps_g1 = ctx.enter_context(tc.tile_pool(name="psg1", bufs=3, space="PSUM"))
        ps_d = ctx.enter_context(tc.tile_pool(name="psd", bufs=1, space="PSUM"))
        ps_s = ctx.enter_context(tc.tile_pool(name="pss", bufs=1, space="PSUM"))

        # DMA issue order is roughly execution order: wdn + block-0
        # activations first (gives PE its runway), then the router means,
        # then wup (trickles in under block 0's GEMM1), then the rest.
        wdn_sb = wpool.tile([128, KD * EH], BF16)       # k-tile k at [:, k*EH:]
        xtb0 = xtp.tile([128, KD * TB], BF16, tag="xtm", name="xtb0")

        def load_xtb(xtb, t0):
            # one 1MB DMA: [128, k, t] <- xt[(k p), t0:t0+TB]
            nc.sync.dma_start(
                out=xtb[:, :].rearrange("p (k t) -> p k t", k=KD),
                in_=xt[:, t0:t0 + TB].rearrange("(k p) t -> p k t", p=128))

        def load_xrb(xrb, t0):
            nc.sync.dma_start(
                out=xrb[:, :].rearrange("p (tt d) -> p tt d", tt=2),
                in_=xres[t0:t0 + TB, :].rearrange("(tt p) d -> p tt d", p=128))

        nc.sync.dma_start(out=wdn_sb[:, 0:EH], in_=wdn[0:128, :])
        load_xtb(xtb0, 0)
        for k in range(1, KD):
            nc.sync.dma_start(out=wdn_sb[:, k * EH:(k + 1) * EH],
                              in_=wdn[k * 128:(k + 1) * 128, :])
        dbt_sb = const.tile([128, NKEH], F32)
        nc.sync.dma_start(out=dbt_sb[:], in_=dbt[:])
        cws_sb = const.tile([128, KD * C], F32)
        for k in range(KD):
            nc.sync.dma_start(out=cws_sb[:, k * C:(k + 1) * C],
                              in_=cws[k * 128:(k + 1) * 128, :])
        cb_sb = const.tile([1, C], F32)
        nc.sync.dma_start(out=cb_sb[:], in_=cb[:])
        sw_sb = const.tile([1, BPC * G], F32)
        nc.sync.dma_start(out=sw_sb[:], in_=sw[:])
        fw0_sb = const.tile([1, E], F32)
        nc.sync.dma_start(out=fw0_sb[:], in_=fw0[:])
        ones_f = const.tile([1, 128], F32)
        nc.any.memset(ones_f[:], 1.0)
        ones_bf = const.tile([1, 128], BF16)
        nc.any.memset(ones_bf[:], 1.0)

        # chunk-0 up-weights early so block-0 GEMM2 isn't starved
        wup_sb = wpool.tile([128, NKEH * DM], FP8 if G2_FP8 else BF16)

        def load_wup(g):                       # one 4-k-tile (1MB bf16) DMA
            nc.sync.dma_start(
                out=wup_sb[:, g * 4 * DM:(g + 1) * 4 * DM].rearrange(
                    "p (k d) -> p k d", k=4),
                in_=wup[g * 512:(g + 1) * 512, :].rearrange(
                    "(k p) d -> p k d", p=128))

        load_wup(0)
        upb_sb = const.tile([E, DM], F32)
        nc.sync.dma_start(out=upb_sb[:], in_=upb[:])

        # ---- feature means + routers, batch-0 first ---------------------
        # half-size [128, S/2] tiles keep the mean pool small; the router
        # matmul simply accumulates 2 partial columns per k-tile
        mf_sb = const.tile([128, BPC * KD * 2], F32)

        def batch_means(b):
            for k in range(KD):
                for h in range(2):
                    xtm = xtp.tile([128, S // 2], BF16, tag="xmean",
                                   name=f"xtm{b}{k}{h}")
                    nc.sync.dma_start(
                        out=xtm[:],
                        in_=xt[k * 128:(k + 1) * 128,
                               b * S + h * (S // 2):b * S + (h + 1) * (S // 2)])
                    col = (b * KD + k) * 2 + h
                    nc.vector.reduce_sum(mf_sb[:, col:col + 1], xtm[:],
                                         axis=AX.X)

        wsb = [None] * BPC
        bias_bf = [None] * BPC

        def batch_router(b):
            if b == 0:
                fw = fw0_sb
                finish_router(b, fw)
                return
            lgp = ps_s.tile([1, C], F32, tag="small")
            for k in range(KD):
                for h in range(2):
                    col = (b * KD + k) * 2 + h
                    nc.tensor.matmul(lgp[:], lhsT=mf_sb[:, col:col + 1],
                                     rhs=cws_sb[:, k * C:(k + 1) * C],
                                     start=(k == 0 and h == 0), stop=False)
            nc.tensor.matmul(lgp[:], lhsT=ones_f[0:1, 0:1], rhs=cb_sb[:],
                             start=False, stop=True)
            lg_sb = const.tile([1, C], F32, tag=f"lg{b}")
            nc.scalar.activation(lg_sb[:], lgp[:], AF.Copy)
            nc.sync.dma_start(out=clg[b:b + 1, :], in_=lg_sb[:])

            e_sb = const.tile([1, C], F32, tag=f"esb{b}")
            nc.scalar.activation(e_sb[:], lg_sb[:], AF.Exp)
            s_sb = const.tile([1, 1], F32, tag=f"ssb{b}")
            nc.vector.reduce_sum(s_sb[:], e_sb[:], axis=AX.X)
            r_sb = const.tile([1, 1], F32, tag=f"rsb{b}")
            nc.vector.reciprocal(r_sb[:], s_sb[:])
            cwd = const.tile([1, C], F32, tag=f"cwd{b}")
            nc.vector.tensor_scalar_mul(cwd[:], e_sb[:], r_sb[0:1, 0:1])

            # flat joint weights fw[e = c*G + g] = cond[c] * stage[g]
            fw = const.tile([1, E], F32, tag=f"fw{b}")
            fw3 = fw[0:1, :].rearrange("p (c g) -> p c g", g=G)
            cwd3 = cwd[0:1, :].rearrange("p (c u) -> p c u", u=1)
            for g in range(G):
                nc.vector.tensor_scalar_mul(fw3[:, :, g:g + 1], cwd3[:],
                                            sw_sb[0:1, b * G + g:b * G + g + 1])

            finish_router(b, fw)

        def finish_router(b, fw):
            # broadcast across partitions: [128, E]
            wbp = ps_s.tile([128, E], F32, tag="small")
            nc.tensor.matmul(wbp[:], lhsT=ones_f[:], rhs=fw[:],
                             start=True, stop=True)
            ws = const.tile([128, E], F32, tag=f"ws{b}")
            nc.scalar.activation(ws[:], wbp[:], AF.Copy,
                                 scale=HS if G2_FP8 else 1.0)
            wsb[b] = ws

            # fw -> column layout [E, 1] via DVE 32x32 transpose
            tin = const.tile([32, 32], F32, tag=f"ti{b}")
            nc.any.memset(tin[:], 0.0)
            nc.vector.tensor_copy(tin[0:1, 0:E], fw[:])
            tco = const.tile([32, 32], F32, tag=f"tc{b}")
            nc.vector.transpose(tco[:], tin[:])

            # bias_term = fw @ up_b   -> [1, DM], cast to bf16
            bb = const.tile([1, DM], BF16, tag=f"bb{b}")
            for hh in range(2):
                bps = ps_s.tile([1, 512], F32, tag="small")
                nc.tensor.matmul(bps[:], lhsT=tco[0:E, 0:1],
                                 rhs=upb_sb[0:E, hh * 512:(hh + 1) * 512],
                                 start=True, stop=True)
                nc.scalar.activation(bb[0:1, hh * 512:(hh + 1) * 512], bps[:],
                                     AF.Copy, scale=TOT if G2_FP8 else 1.0)
            bias_bf[b] = bb

        # remaining up-weights trickle in behind block 0's GEMM1 prologue
        for g in range(1, NKEH // 4):
            load_wup(g)
        xrb0 = xrp.tile([128, 2 * DM], F32, name="xrb0", tag="xrb")
        load_xrb(xrb0, 0)

        # ---- main pipeline ----------------------------------------------
        def emit_g1(xtb, htc, c):
            """GEMM1 for one EH chunk + fused gelu eviction (unscaled)."""
            for m in range(4):
                g1 = ps_g1.tile([128, TB], F32, tag="g1", name="g1")
                col0 = c * CHW + m * 128
                for k in range(KD):
                    nc.tensor.matmul(
                        g1[:],
                        lhsT=wdn_sb[:, k * EH + col0:k * EH + col0 + 128],
                        rhs=xtb[:, k * TB:(k + 1) * TB],
                        start=(k == 0), stop=(k == KD - 1))
                eh_t = c * 4 + m
                nc.scalar.activation(htc[:, m * TB:(m + 1) * TB], g1[:],
                                     AF.Gelu, bias=dbt_sb[:, eh_t:eh_t + 1])

        def emit_scale(htc, hdst, c, b):
            """Per-(batch, expert) routing-weight scale (+fp8 cast)."""
            for m in range(4):
                e_idx = (c * CHW + m * 128) // HID
                nc.vector.tensor_scalar_mul(
                    hdst[:, m * TB:(m + 1) * TB], htc[:, m * TB:(m + 1) * TB],
                    wsb[b][:, e_idx:e_idx + 1])

        def emit_g2(htc, dps, c):
            if G2_FP8:
                ht3 = htc[:, :].rearrange("p (m t) -> p m t", m=4)
                wu3 = wup_sb[:, :].rearrange("p (kk dd) -> p kk dd", kk=NKEH)
                for tt in range(2):
                    for m in (0, 2):
                        kk = c * 4 + m
                        for hh in range(2):
                            nc.tensor.matmul(
                                dps[tt * 2 + hh][:],
                                lhsT=ht3[:, m:m + 2, tt * 128:(tt + 1) * 128],
                                rhs=wu3[:, kk:kk + 2, hh * 512:(hh + 1) * 512],
                                start=(c == 0 and m == 0), stop=False,
                                perf_mode=mybir.MatmulPerfMode.DoubleRow)
            else:
                for tt in range(2):
                    for m in range(4):
                        lhs = htc[:, m * TB + tt * 128:m * TB + (tt + 1) * 128]
                        kk = c * 4 + m
                        for hh in range(2):
                            nc.tensor.matmul(
                                dps[tt * 2 + hh][:], lhsT=lhs,
                                rhs=wup_sb[:, kk * DM + hh * 512:
                                           kk * DM + (hh + 1) * 512],
                                start=(c == 0 and m == 0), stop=False)

        def emit_tail(dps, xrb, t0, b):
            # up_b bias term closes each accumulation group; eviction adds
            # the fp32 residual
            outb = outp.tile([128, 2 * DM], F32, name="outb")
            for tt in range(2):
                for hh in range(2):
                    nc.tensor.matmul(
                        dps[tt * 2 + hh][:], lhsT=ones_bf[:],
                        rhs=bias_bf[b][0:1, hh * 512:(hh + 1) * 512],
                        start=False, stop=True)
                    sl = slice(tt * DM + hh * 512, tt * DM + (hh + 1) * 512)
                    if G2_FP8:
                        nc.vector.tensor_tensor(
                            out=dps[tt * 2 + hh][:], in0=dps[tt * 2 + hh][:],
                            in1=xrb[:, sl], op=ALU.add)
                        nc.scalar.activation(outb[:, sl], dps[tt * 2 + hh][:],
                                             AF.Copy, scale=1.0 / TOT)
                    else:
                        nc.vector.tensor_tensor(
                            out=outb[:, sl], in0=dps[tt * 2 + hh][:],
                            in1=xrb[:, sl], op=ALU.add)
            nc.sync.dma_start(
                out=out[t0:t0 + TB, :].rearrange("(tt p) d -> p tt d", p=128),
                in_=outb[:, :].rearrange("p (tt d) -> p tt d", tt=2))

        HDT = FP8 if G2_FP8 else BF16

        # block (0,0) prologue: all of GEMM1 runs before the router in PE
        # order, so the PE never waits on the means DMAs; scale + GEMM2
        # follow once the router lands.
        dps = [ps_d.tile([128, 512], F32, tag=f"d{i}", name=f"dps{i}")
               for i in range(4)]
        htcs0 = [htp.tile([128, 4 * TB], HDT, name=f"htc0c{c}",
                          tag=f"h{c}", bufs=1)
                 for c in range(NCH)]
        for c in range(NCH):
            emit_g1(xtb0, htcs0[c], c)
        batch_router(0)
        for c in range(NCH):
            emit_scale(htcs0[c], htcs0[c], c, 0)
            emit_g2(htcs0[c], dps, c)
        emit_tail(dps, xrb0, 0, 0)

        for b in range(BPC):
            for blk in range(NBLK):
                if b == 0 and blk == 0:
                    continue
                if b == 0 and blk == 2:
                    # batch-1 means DMAs queue behind block 0/1 loads; its
                    # router matmuls are emitted at block 6 so the PE never
                    # waits on them in-order
                    batch_means(1)
                if b == 0 and blk == 6:
                    batch_router(1)
                t0 = b * S + blk * TB
                xtb = xtp.tile([128, KD * TB], BF16, tag="xtm", name="xtb")
                load_xtb(xtb, t0)
                xrb = xrp.tile([128, 2 * DM], F32, name="xrb", tag="xrb")
                load_xrb(xrb, t0)

                dps = [ps_d.tile([128, 512], F32, tag=f"d{i}", name=f"dps{i}")
                       for i in range(4)]
                for c in range(NCH):
                    htc = htp.tile([128, 4 * TB], HDT, name="htc",
                                   tag=f"h{c}", bufs=1)
                    emit_g1(xtb, htc, c)
                    emit_scale(htc, htc, c, b)
                    emit_g2(htc, dps, c)
                emit_tail(dps, xrb, t0, b)

    nc.compile()
    return nc


def _get_nc():
    if "nc" not in _CACHE:
        _CACHE["nc"] = _build()
    return _CACHE["nc"]


def _softmax32(x):
    x = np.asarray(x, np.float32)
    m = x.max(axis=-1, keepdims=True)
    e = np.exp(x - m)
    return e / e.sum(axis=-1, keepdims=True)


def kernel(**inputs):
    global LAST_RESULT
    features = np.ascontiguousarray(np.asarray(inputs["features"], np.float32))
    x_raw = np.asarray(inputs["x_raw"], np.float32)
    down_w = np.asarray(inputs["down_w"], np.float32)
    down_b = np.asarray(inputs["down_b"], np.float32)
    up_w = np.asarray(inputs["up_w"], np.float32)
    up_b = np.asarray(inputs["up_b"], np.float32)
    cond_w = np.asarray(inputs["cond_w"], np.float32)
    cond_b = np.asarray(inputs["cond_b"], np.float32)
    stage_w = np.asarray(inputs["stage_w"], np.float32)
    stage_b = np.asarray(inputs["stage_b"], np.float32)

    # host-side router for the tiny stage branch (16x2048x14 input)
    stage_logits = x_raw.mean(axis=1, dtype=np.float32) @ stage_w + stage_b
    stage_weights = _softmax32(stage_logits)            # [B, G]

    wdn_np = np.ascontiguousarray(
        down_w.transpose(1, 0, 2).reshape(DM, EH)).astype(BF16_NP)
    if G2_FP8:
        wup_np = (up_w.reshape(EH, DM) * np.float32(WS)).astype(FP8_NP)
    else:
        wup_np = np.ascontiguousarray(up_w.reshape(EH, DM)).astype(BF16_NP)
    dbt_np = np.ascontiguousarray(down_b.reshape(EH).reshape(NKEH, 128).T)
    upb_np = np.ascontiguousarray(up_b)                  # [E, DM]
    cws_np = np.ascontiguousarray(cond_w / np.float32(S))
    cb_np = cond_b.reshape(1, C)

    # batch-0-of-each-core cond router on host (pipeline warm-up: keeps the
    # device's first block off the means/router critical path; ~0.02% of the
    # model FLOPs). Odd batches are routed on-device.
    b0_idx = np.arange(0, B, BPC)
    mean_f0 = features[b0_idx].mean(axis=1, dtype=np.float32)      # [8, DM]
    logits0 = mean_f0 @ cond_w + cond_b
    cond0 = _softmax32(logits0)                                     # [8, C]

    nc = _get_nc()
    in_maps = []
    for cidx in range(NCORE):
        fs = features[BPC * cidx:BPC * (cidx + 1)].reshape(TPC, DM)
        flat0 = (cond0[cidx][:, None] *
                 stage_weights[BPC * cidx][None, :]).reshape(1, E)
        in_maps.append({
            "xt": fs.T.astype(BF16_NP),
            "xres": fs * np.float32(TOT) if G2_FP8 else fs,
            "wdn": wdn_np,
            "wup": wup_np,
            "dbt": dbt_np,
            "upb": upb_np,
            "cws": cws_np,
            "cb": cb_np,
            "sw": np.ascontiguousarray(
                stage_weights[BPC * cidx:BPC * (cidx + 1)].reshape(1, BPC * G)),
            "fw0": np.ascontiguousarray(flat0, dtype=np.float32),
        })

    res = run_bass_kernel_spmd(nc, in_maps, list(range(NCORE)))
    LAST_RESULT = res

    output = np.concatenate(
        [res.results[c]["out"] for c in range(NCORE)], axis=0
    ).reshape(B, S, DM)
    cond_weights = np.empty((B, C), np.float32)
    cond_weights[b0_idx] = cond0
    for c in range(NCORE):
        cond_weights[BPC * c + 1] = _softmax32(res.results[c]["clg"][1])

    joint = cond_weights[:, :, None] * stage_weights[:, None, :]
    flat = joint.reshape(B, E).astype(np.float32)
    expert_loads = flat.mean(axis=0, dtype=np.float32)
    lb_loss = np.float32(E * np.sum(expert_loads * expert_loads,
                                    dtype=np.float32) * np.float32(0.01))

    return (output.astype(np.float32), cond_weights.astype(np.float32),
            stage_weights.astype(np.float32), expert_loads, lb_loss)


# revision 37
# speedup vs baseline: 1.0752x; 1.0016x over previous
"""Trainium2 Bass kernel for the DSA-MoE routing module.

Strategy: data-parallel over batch. Each of the 8 NeuronCores gets 2 full
batches (4096 tokens). Expert weights are replicated, cast to bf16 on host,
and kept SBUF-resident. Per core:

  - cond router: sum features over seq via DVE reduce on the pre-transposed
    bf16 activations, then a small fp32 matmul against cond_w/2048 (+cond_b
    via a rank-1 matmul). Final softmax happens on host from the returned
    logits; an on-device softmax produces the internal routing weights.
  - stage router: x_raw is tiny (16x2048x14) -> host computes stage softmax
    and passes the per-batch weights in.
  - main pipeline per 256-token block: GEMM1 (X.T @ W_down -> H.T in
    [EH, tokens] layout, bf16, PSUM fp32), fused Gelu+down_b eviction on
    ACT, per-(batch,expert) scale on DVE, GEMM2 accumulates
    delta = Hw.T^T @ W_up directly in PSUM across all 9 EH-chunks
    (plus a rank-1 matmul adding the up_b bias term), final eviction adds
    the fp32 residual.

The full (unsharded) inputs come in; sharding/gather happens on host.
"""

import sys

sys.path.insert(0, "/opt/trn_rl_repo")

from contextlib import ExitStack

import ml_dtypes
import numpy as np

import concourse.bass as bass  # noqa: F401  (registers bass types)
import concourse.tile as tile
from concourse import bacc, mybir
from concourse.bass_utils import run_bass_kernel_spmd

# If BASS_TRACE is set but this image's antenv lacks the axon NTFF hook
# module, bass_utils would die on import. Register a no-op fallback so
# tracing degrades gracefully instead (a real hook, e.g. from test.py,
# takes precedence since this only fills in a missing module).
try:
    import antenv.axon_hooks  # noqa: F401
except ImportError:
    import types as _types

    import antenv as _antenv

    _hooks = _types.ModuleType("antenv.axon_hooks")
    _hooks._hook = None
    _hooks.set_axon_ntff_profile_hook = (
        lambda h: setattr(_hooks, "_hook", h))
    _hooks.get_axon_ntff_profile_hook = lambda: _hooks._hook
    sys.modules["antenv.axon_hooks"] = _hooks
    _antenv.axon_hooks = _hooks

BF16, F32 = mybir.dt.bfloat16, mybir.dt.float32
FP8 = mybir.dt.float8e4
AF = mybir.ActivationFunctionType
AX = mybir.AxisListType
ALU = mybir.AluOpType
BF16_NP = ml_dtypes.bfloat16
FP8_NP = ml_dtypes.float8_e4m3

# fp8 GEMM2: h is pre-scaled by HS (off the e4m3 subnormal range) and W_up by
# WS; the final eviction divides the PSUM result by HS*WS, with the residual
# and bias terms pre-multiplied to match.
G2_FP8 = False
HS = 128.0
WS = 64.0
TOT = HS * WS

B, S, DM, HID = 16, 2048, 1024, 256
C, G = 6, 3
E = C * G                      # 18 experts
EH = E * HID                   # 4608
NCORE = 8
BPC = B // NCORE               # batches per core = 2
TPC = BPC * S                  # tokens per core = 4096
TB = 256                       # token block
NBLK = S // TB                 # blocks per batch = 8
KD = DM // 128                 # 8 k-tiles over D
NCH = 9                        # EH chunks
CHW = EH // NCH                # 512 EH cols per chunk
NKEH = EH // 128               # 36 EH k-tiles

_CACHE = {}
LAST_RESULT = None


def _build():
    nc = bacc.Bacc("TRN2", target_bir_lowering=False, debug=False,
                   num_devices=NCORE)
    xt = nc.dram_tensor("xt", [DM, TPC], BF16, kind="ExternalInput").ap()
    xres = nc.dram_tensor("xres", [TPC, DM], F32, kind="ExternalInput").ap()
    wdn = nc.dram_tensor("wdn", [DM, EH], BF16, kind="ExternalInput").ap()
    wup = nc.dram_tensor("wup", [EH, DM], FP8 if G2_FP8 else BF16,
                         kind="ExternalInput").ap()
    upb = nc.dram_tensor("upb", [E, DM], F32, kind="ExternalInput").ap()
    # packed small consts: [:, 0:36]=down_b.T tiles, [36:84]=cond_w/S per
    # k-chunk, row 0 of [84:90]=cond_b, [90:96]=stage weights, [96:114]=fw0
    cpk = nc.dram_tensor("cpk", [128, 114], F32, kind="ExternalInput").ap()
    out = nc.dram_tensor("out", [TPC, DM], F32, kind="ExternalOutput").ap()
    clg = nc.dram_tensor("clg", [BPC, C], F32, kind="ExternalOutput").ap()

    with tile.TileContext(nc) as tc, ExitStack() as ctx:
        const = ctx.enter_context(tc.tile_pool(name="const", bufs=1))
        wpool = ctx.enter_context(tc.tile_pool(name="wpool", bufs=1))
        xtp = ctx.enter_context(tc.tile_pool(name="xtp", bufs=2))
        xrp = ctx.enter_context(tc.tile_pool(name="xrp", bufs=1))
        htp = ctx.enter_context(tc.tile_pool(name="htp", bufs=1))
        outp = ctx.enter_context(tc.tile_pool(name="outp", bufs=1))
        ps_g1 = ctx.enter_context(tc.tile_pool(name="psg1", bufs=3, space="PSUM"))
        ps_d = ctx.enter_context(tc.tile_pool(name="psd", bufs=1, space="PSUM"))
        ps_s = ctx.enter_context(tc.tile_pool(name="pss", bufs=1, space="PSUM"))

        # DMA issue order is roughly execution order: wdn + block-0
        # activations first (gives PE its runway), then the router means,
        # then wup (trickles in under block 0's GEMM1), then the rest.
        wdn_sb = wpool.tile([128, KD * EH], BF16)       # k-tile k at [:, k*EH:]
        xtb0 = xtp.tile([128, KD * TB], BF16, tag="xtm", name="xtb0")

        def load_xtb(xtb, t0):
            # one 1MB DMA: [128, k, t] <- xt[(k p), t0:t0+TB]
            nc.sync.dma_start(
                out=xtb[:, :].rearrange("p (k t) -> p k t", k=KD),
                in_=xt[:, t0:t0 + TB].rearrange("(k p) t -> p k t", p=128))

        def load_xrb(xrb, t0):
            nc.sync.dma_start(
                out=xrb[:, :].rearrange("p (tt d) -> p tt d", tt=2),
                in_=xres[t0:t0 + TB, :].rearrange("(tt p) d -> p tt d", p=128))

        nc.sync.dma_start(out=wdn_sb[:, 0:EH], in_=wdn[0:128, :])
        nc.scalar.dma_start(
            out=xtb0[:, :].rearrange("p (k t) -> p k t", k=KD),
            in_=xt[:, 0:TB].rearrange("(k p) t -> p k t", p=128))
        for k in range(1, KD):
            eng = nc.scalar if k % 2 else nc.sync
            eng.dma_start(out=wdn_sb[:, k * EH:(k + 1) * EH],
                          in_=wdn[k * 128:(k + 1) * 128, :])
        cpk_sb = const.tile([128, 114], F32)
        nc.sync.dma_start(out=cpk_sb[:], in_=cpk[:])
        dbt_sb = cpk_sb[:, 0:36]
        cws_sb = cpk_sb[:, 36:84]
        cb_sb = cpk_sb[0:1, 84:90]
        sw_sb = cpk_sb[0:1, 90:96]
        fw0_sb = cpk_sb[0:1, 96:114]
        ones_f = const.tile([1, 128], F32)
        nc.any.memset(ones_f[:], 1.0)
        ones_bf = const.tile([1, 128], BF16)
        nc.any.memset(ones_bf[:], 1.0)

        # chunk-0 up-weights early so block-0 GEMM2 isn't starved
        wup_sb = wpool.tile([128, NKEH * DM], FP8 if G2_FP8 else BF16)

        def load_wup(g):                       # one 4-k-tile (1MB bf16) DMA
            nc.sync.dma_start(
                out=wup_sb[:, g * 4 * DM:(g + 1) * 4 * DM].rearrange(
                    "p (k d) -> p k d", k=4),
                in_=wup[g * 512:(g + 1) * 512, :].rearrange(
                    "(k p) d -> p k d", p=128))

        load_wup(0)
        upb_sb = const.tile([E, DM], F32)
        nc.sync.dma_start(out=upb_sb[:], in_=upb[:])

        # ---- feature means + routers, batch-0 first ---------------------
        # half-size [128, S/2] tiles keep the mean pool small; the router
        # matmul simply accumulates 2 partial columns per k-tile
        mf_sb = const.tile([128, BPC * KD * 2], F32)

        def batch_means(b):
            for k in range(KD):
                for h in range(2):
                    xtm = xtp.tile([128, S // 2], BF16, tag="xmean",
                                   name=f"xtm{b}{k}{h}")
                    nc.sync.dma_start(
                        out=xtm[:],
                        in_=xt[k * 128:(k + 1) * 128,
                               b * S + h * (S // 2):b * S + (h + 1) * (S // 2)])
                    col = (b * KD + k) * 2 + h
                    nc.vector.reduce_sum(mf_sb[:, col:col + 1], xtm[:],
                                         axis=AX.X)

        wsb = [None] * BPC
        bias_bf = [None] * BPC

        def batch_router(b):
            if b == 0:
                fw = fw0_sb
                finish_router(b, fw)
                return
            lgp = ps_s.tile([1, C], F32, tag="small")
            for k in range(KD):
                for h in range(2):
                    col = (b * KD + k) * 2 + h
                    nc.tensor.matmul(lgp[:], lhsT=mf_sb[:, col:col + 1],
                                     rhs=cws_sb[:, k * C:(k + 1) * C],
                                     start=(k == 0 and h == 0), stop=False)
            nc.tensor.matmul(lgp[:], lhsT=ones_f[0:1, 0:1], rhs=cb_sb[:],
                             start=False, stop=True)
            lg_sb = const.tile([1, C], F32, tag=f"lg{b}")
            nc.scalar.activation(lg_sb[:], lgp[:], AF.Copy)
            nc.sync.dma_start(out=clg[b:b + 1, :], in_=lg_sb[:])

            e_sb = const.tile([1, C], F32, tag=f"esb{b}")
            nc.scalar.activation(e_sb[:], lg_sb[:], AF.Exp)
            s_sb = const.tile([1, 1], F32, tag=f"ssb{b}")
            nc.vector.reduce_sum(s_sb[:], e_sb[:], axis=AX.X)
            r_sb = const.tile([1, 1], F32, tag=f"rsb{b}")
            nc.vector.reciprocal(r_sb[:], s_sb[:])
            cwd = const.tile([1, C], F32, tag=f"cwd{b}")
            nc.vector.tensor_scalar_mul(cwd[:], e_sb[:], r_sb[0:1, 0:1])

            # flat joint weights fw[e = c*G + g] = cond[c] * stage[g]
            fw = const.tile([1, E], F32, tag=f"fw{b}")
            fw3 = fw[0:1, :].rearrange("p (c g) -> p c g", g=G)
            cwd3 = cwd[0:1, :].rearrange("p (c u) -> p c u", u=1)
            for g in range(G):
                nc.vector.tensor_scalar_mul(fw3[:, :, g:g + 1], cwd3[:],
                                            sw_sb[0:1, b * G + g:b * G + g + 1])

            finish_router(b, fw)

        def finish_router(b, fw):
            # broadcast across partitions: [128, E]
            wbp = ps_s.tile([128, E], F32, tag="small")
            nc.tensor.matmul(wbp[:], lhsT=ones_f[:], rhs=fw[:],
                             start=True, stop=True)
            ws = const.tile([128, E], F32, tag=f"ws{b}")
            nc.scalar.activation(ws[:], wbp[:], AF.Copy,
                                 scale=HS if G2_FP8 else 1.0)
            wsb[b] = ws

            # fw -> column layout [E, 1] via DVE 32x32 transpose
            tin = const.tile([32, 32], F32, tag=f"ti{b}")
            nc.any.memset(tin[:], 0.0)
            nc.vector.tensor_copy(tin[0:1, 0:E], fw[:])
            tco = const.tile([32, 32], F32, tag=f"tc{b}")
            nc.vector.transpose(tco[:], tin[:])

            # bias_term = fw @ up_b   -> [1, DM], cast to bf16
            bb = const.tile([1, DM], BF16, tag=f"bb{b}")
            for hh in range(2):
                bps = ps_s.tile([1, 512], F32, tag="small")
                nc.tensor.matmul(bps[:], lhsT=tco[0:E, 0:1],
                                 rhs=upb_sb[0:E, hh * 512:(hh + 1) * 512],
                                 start=True, stop=True)
                nc.scalar.activation(bb[0:1, hh * 512:(hh + 1) * 512], bps[:],
                                     AF.Copy, scale=TOT if G2_FP8 else 1.0)
            bias_bf[b] = bb

        # remaining up-weights trickle in behind block 0's GEMM1 prologue
        for g in range(1, NKEH // 4):
            load_wup(g)
        xrb0 = xrp.tile([128, 2 * DM], F32, name="xrb0", tag="xrb")
        load_xrb(xrb0, 0)

        # ---- main pipeline ----------------------------------------------
        def emit_g1(xtb, htc, c):
            """GEMM1 for one EH chunk + fused gelu eviction (unscaled)."""
            for m in range(4):
                g1 = ps_g1.tile([128, TB], F32, tag="g1", name="g1")
                col0 = c * CHW + m * 128
                for k in range(KD):
                    nc.tensor.matmul(
                        g1[:],
                        lhsT=wdn_sb[:, k * EH + col0:k * EH + col0 + 128],
                        rhs=xtb[:, k * TB:(k + 1) * TB],
                        start=(k == 0), stop=(k == KD - 1))
                eh_t = c * 4 + m
                nc.scalar.activation(htc[:, m * TB:(m + 1) * TB], g1[:],
                                     AF.Gelu, bias=dbt_sb[:, eh_t:eh_t + 1])

        def emit_scale(htc, hdst, c, b):
            """Per-(batch, expert) routing-weight scale (+fp8 cast)."""
            for m in range(4):
                e_idx = (c * CHW + m * 128) // HID
                nc.vector.tensor_scalar_mul(
                    hdst[:, m * TB:(m + 1) * TB], htc[:, m * TB:(m + 1) * TB],
                    wsb[b][:, e_idx:e_idx + 1])

        def emit_g2(htc, dps, c):
            if G2_FP8:
                ht3 = htc[:, :].rearrange("p (m t) -> p m t", m=4)
                wu3 = wup_sb[:, :].rearrange("p (kk dd) -> p kk dd", kk=NKEH)
                for tt in range(2):
                    for m in (0, 2):
                        kk = c * 4 + m
                        for hh in range(2):
                            nc.tensor.matmul(
                                dps[tt * 2 + hh][:],
                                lhsT=ht3[:, m:m + 2, tt * 128:(tt + 1) * 128],
                                rhs=wu3[:, kk:kk + 2, hh * 512:(hh + 1) * 512],
                                start=(c == 0 and m == 0), stop=False,
                                perf_mode=mybir.MatmulPerfMode.DoubleRow)
            else:
                for tt in range(2):
                    for m in range(4):
                        lhs = htc[:, m * TB + tt * 128:m * TB + (tt + 1) * 128]
                        kk = c * 4 + m
                        for hh in range(2):
                            nc.tensor.matmul(
                                dps[tt * 2 + hh][:], lhsT=lhs,
                                rhs=wup_sb[:, kk * DM + hh * 512:
                                           kk * DM + (hh + 1) * 512],
                                start=(c == 0 and m == 0), stop=False)

        def emit_tail(dps, xrb, t0, b):
            # up_b bias term closes each accumulation group; eviction adds
            # the fp32 residual
            outb = outp.tile([128, 2 * DM], F32, name="outb")
            for tt in range(2):
                for hh in range(2):
                    nc.tensor.matmul(
                        dps[tt * 2 + hh][:], lhsT=ones_bf[:],
                        rhs=bias_bf[b][0:1, hh * 512:(hh + 1) * 512],
                        start=False, stop=True)
                    sl = slice(tt * DM + hh * 512, tt * DM + (hh + 1) * 512)
                    if G2_FP8:
                        nc.vector.tensor_tensor(
                            out=dps[tt * 2 + hh][:], in0=dps[tt * 2 + hh][:],
                            in1=xrb[:, sl], op=ALU.add)
                        nc.scalar.activation(outb[:, sl], dps[tt * 2 + hh][:],
                                             AF.Copy, scale=1.0 / TOT)
                    else:
                        nc.vector.tensor_tensor(
                            out=outb[:, sl], in0=dps[tt * 2 + hh][:],
                            in1=xrb[:, sl], op=ALU.add)
            nc.sync.dma_start(
                out=out[t0:t0 + TB, :].rearrange("(tt p) d -> p tt d", p=128),
                in_=outb[:, :].rearrange("p (tt d) -> p tt d", tt=2))

        HDT = FP8 if G2_FP8 else BF16

        # block (0,0) prologue: all of GEMM1 runs before the router in PE
        # order, so the PE never waits on the means DMAs; scale + GEMM2
        # follow once the router lands.
        dps = [ps_d.tile([128, 512], F32, tag=f"d{i}", name=f"dps{i}")
               for i in range(4)]
        htcs0 = [htp.tile([128, 4 * TB], HDT, name=f"htc0c{c}",
                          tag=f"h{c}", bufs=1)
                 for c in range(NCH)]
        for c in range(NCH):
            emit_g1(xtb0, htcs0[c], c)
        batch_router(0)
        for c in range(NCH):
            emit_scale(htcs0[c], htcs0[c], c, 0)
            emit_g2(htcs0[c], dps, c)
        emit_tail(dps, xrb0, 0, 0)

        for b in range(BPC):
            for blk in range(NBLK):
                if b == 0 and blk == 0:
                    continue
                if b == 0 and blk == 2:
                    # batch-1 means DMAs queue behind block 0/1 loads; its
                    # router matmuls are emitted at block 6 so the PE never
                    # waits on them in-order
                    batch_means(1)
                if b == 0 and blk == 6:
                    batch_router(1)
                t0 = b * S + blk * TB
                xtb = xtp.tile([128, KD * TB], BF16, tag="xtm", name="xtb")
                load_xtb(xtb, t0)
                xrb = xrp.tile([128, 2 * DM], F32, name="xrb", tag="xrb")
                load_xrb(xrb, t0)

                dps = [ps_d.tile([128, 512], F32, tag=f"d{i}", name=f"dps{i}")
                       for i in range(4)]
                for c in range(NCH):
                    htc = htp.tile([128, 4 * TB], HDT, name="htc",
                                   tag=f"h{c}", bufs=1)
                    emit_g1(xtb, htc, c)
                    emit_scale(htc, htc, c, b)
                    emit_g2(htc, dps, c)
                emit_tail(dps, xrb, t0, b)

    nc.compile()
    return nc


def _get_nc():
    if "nc" not in _CACHE:
        _CACHE["nc"] = _build()
    return _CACHE["nc"]


def _softmax32(x):
    x = np.asarray(x, np.float32)
    m = x.max(axis=-1, keepdims=True)
    e = np.exp(x - m)
    return e / e.sum(axis=-1, keepdims=True)


def kernel(**inputs):
    global LAST_RESULT
    features = np.ascontiguousarray(np.asarray(inputs["features"], np.float32))
    x_raw = np.asarray(inputs["x_raw"], np.float32)
    down_w = np.asarray(inputs["down_w"], np.float32)
    down_b = np.asarray(inputs["down_b"], np.float32)
    up_w = np.asarray(inputs["up_w"], np.float32)
    up_b = np.asarray(inputs["up_b"], np.float32)
    cond_w = np.asarray(inputs["cond_w"], np.float32)
    cond_b = np.asarray(inputs["cond_b"], np.float32)
    stage_w = np.asarray(inputs["stage_w"], np.float32)
    stage_b = np.asarray(inputs["stage_b"], np.float32)

    # host-side router for the tiny stage branch (16x2048x14 input)
    stage_logits = x_raw.mean(axis=1, dtype=np.float32) @ stage_w + stage_b
    stage_weights = _softmax32(stage_logits)            # [B, G]

    wdn_np = np.ascontiguousarray(
        down_w.transpose(1, 0, 2).reshape(DM, EH)).astype(BF16_NP)
    cpk_np = np.zeros((128, 114), np.float32)
    cpk_np[:, 0:36] = down_b.reshape(EH).reshape(NKEH, 128).T
    cpk_np[:, 36:84] = (cond_w / np.float32(S)).reshape(KD, 128, C
                                                        ).transpose(1, 0, 2
                                                        ).reshape(128, KD * C)
    cpk_np[0, 84:90] = cond_b
    if G2_FP8:
        wup_np = (up_w.reshape(EH, DM) * np.float32(WS)).astype(FP8_NP)
    else:
        wup_np = np.ascontiguousarray(up_w.reshape(EH, DM)).astype(BF16_NP)
    upb_np = np.ascontiguousarray(up_b)                  # [E, DM]

    # batch-0-of-each-core cond router on host (pipeline warm-up: keeps the
    # device's first block off the means/router critical path; ~0.02% of the
    # model FLOPs). Odd batches are routed on-device.
    b0_idx = np.arange(0, B, BPC)
    mean_f0 = features[b0_idx].mean(axis=1, dtype=np.float32)      # [8, DM]
    logits0 = mean_f0 @ cond_w + cond_b
    cond0 = _softmax32(logits0)                                     # [8, C]

    nc = _get_nc()
    in_maps = []
    for cidx in range(NCORE):
        fs = features[BPC * cidx:BPC * (cidx + 1)].reshape(TPC, DM)
        flat0 = (cond0[cidx][:, None] *
                 stage_weights[BPC * cidx][None, :]).reshape(E)
        cpk_c = cpk_np.copy()
        cpk_c[0, 90:96] = stage_weights[BPC * cidx:BPC * (cidx + 1)].reshape(
            BPC * G)
        cpk_c[0, 96:114] = flat0
        in_maps.append({
            "xt": fs.T.astype(BF16_NP),
            "xres": fs * np.float32(TOT) if G2_FP8 else fs,
            "wdn": wdn_np,
            "wup": wup_np,
            "upb": upb_np,
            "cpk": cpk_c,
        })

    res = run_bass_kernel_spmd(nc, in_maps, list(range(NCORE)))
    LAST_RESULT = res

    output = np.concatenate(
        [res.results[c]["out"] for c in range(NCORE)], axis=0
    ).reshape(B, S, DM)
    cond_weights = np.empty((B, C), np.float32)
    cond_weights[b0_idx] = cond0
    for c in range(NCORE):
        cond_weights[BPC * c + 1] = _softmax32(res.results[c]["clg"][1])

    joint = cond_weights[:, :, None] * stage_weights[:, None, :]
    flat = joint.reshape(B, E).astype(np.float32)
    expert_loads = flat.mean(axis=0, dtype=np.float32)
    lb_loss = np.float32(E * np.sum(expert_loads * expert_loads,
                                    dtype=np.float32) * np.float32(0.01))

    return (output.astype(np.float32), cond_weights.astype(np.float32),
            stage_weights.astype(np.float32), expert_loads, lb_loss)
